# revision 1
# baseline (speedup 1.0000x reference)
"""Tensor-parallel attention kernel for TRN2 (8 NeuronCores).

Sharding: 2D grid — 4 batches x 2 head-groups (8 heads each). Core c handles
batch b = c // 2, head-group g = c % 2.  Each core:
  1. QK^T projection (transposed layout [dh, s]) with RoPE fused at PSUM
     eviction (partition-crossing DVE multiplies straight from PSUM).
  2. V projection in [s, dh] layout.
  3. Per-head attention, scores kept transposed [t, s] so every contraction
     is layout-native; softmax denominator ridealong via a ones-matmul into
     its own PSUM bank; exp on ACT with the 1/sqrt(dh) scale folded in.
  4. Output projection against the w_o row shard -> partial output.
Host sums the two head-group partials per batch (the "all-reduce" of the
tensor-parallel hint, done in the unshard step), so no device collectives.

All matmuls f16 with fp32 PSUM accumulation.
"""

from contextlib import ExitStack

import numpy as np

B, SQ, SKV = 4, 1024, 1024
D_MODEL = 2048
N_HEADS = 16
D_HEAD = 128
ROPE_THETA = 10000.0
N_CORES = 8
HG = 8  # heads per core
P = 128

F16 = np.float16

_BUILD_CACHE = {}


def _rope_tables():
    """cosf/sinn [128, 1024] f32 for new-token positions offset + s."""
    inv_freq = 1.0 / (ROPE_THETA ** (np.arange(0, D_HEAD, 2, dtype=np.float32) / D_HEAD))
    pos = (SKV + np.arange(SQ, dtype=np.float32))
    ang = pos[:, None] * inv_freq[None, :]           # [S, 64]
    cos = np.cos(ang).astype(np.float32).T           # [64, S]
    sin = np.sin(ang).astype(np.float32).T
    cosf = np.concatenate([cos, cos], axis=0)        # [128, S]
    sinn = np.concatenate([-sin, sin], axis=0)       # [128, S]
    return np.ascontiguousarray(cosf), np.ascontiguousarray(sinn)


def build_module():
    import concourse.mybir as mybir
    import concourse.tile as tile
    from concourse import bacc
    from concourse.bass import ts

    f32 = mybir.dt.float32
    f16 = mybir.dt.float16

    nc = bacc.Bacc("TRN2", target_bir_lowering=False, debug=False,
                   num_devices=N_CORES)

    d_xt = nc.dram_tensor("xt", [P, 16, SQ], f16, kind="ExternalInput").ap()
    d_wqk = nc.dram_tensor("wqk", [P, 16, 2048], f16, kind="ExternalInput").ap()
    d_wv = nc.dram_tensor("wv", [P, 16, 1024], f16, kind="ExternalInput").ap()
    d_ck = nc.dram_tensor("ck", [P, HG, SKV], f16, kind="ExternalInput").ap()
    d_cv = nc.dram_tensor("cv", [P, HG, 8, D_HEAD], f16, kind="ExternalInput").ap()
    d_wo = nc.dram_tensor("wo", [P, HG, 2048], f16, kind="ExternalInput").ap()
    d_cos = nc.dram_tensor("cosf", [P, SQ], f32, kind="ExternalInput").ap()
    d_sin = nc.dram_tensor("sinn", [P, SQ], f32, kind="ExternalInput").ap()
    d_out = nc.dram_tensor("out", [SQ, D_MODEL], f32, kind="ExternalOutput").ap()

    EXP = mybir.ActivationFunctionType.Exp
    MUL = mybir.AluOpType.mult
    ADD = mybir.AluOpType.add
    DIV = mybir.AluOpType.divide
    SCALE = float(D_HEAD) ** -0.5

    with tile.TileContext(nc) as tc, ExitStack() as ctx:
        const = ctx.enter_context(tc.tile_pool(name="const", bufs=1))
        resident = ctx.enter_context(tc.tile_pool(name="res", bufs=1))
        wqk_pool = ctx.enter_context(tc.tile_pool(name="wqk", bufs=3))
        wst_pool = ctx.enter_context(tc.tile_pool(name="wst", bufs=6))
        swap_pool = ctx.enter_context(tc.tile_pool(name="swap", bufs=2))
        tmp_pool = ctx.enter_context(tc.tile_pool(name="tmp", bufs=4))
        exp_pool = ctx.enter_context(tc.tile_pool(name="exp", bufs=8))
        recip_pool = ctx.enter_context(tc.tile_pool(name="recip", bufs=1))
        og_pool = ctx.enter_context(tc.tile_pool(name="og", bufs=5))
        # one unified PSUM pool: 4 tags x [128,1024] (2 banks each) = all 8 banks
        pp = ctx.enter_context(tc.tile_pool(name="pp", bufs=1, space="PSUM"))

        def ptile(tag, name):
            return pp.tile([P, SQ], f32, tag=tag, name=name)

        def ptile1(tag, name):
            return pp.tile([P, 512], f32, tag=tag, name=name)

        # ---- resident loads ----
        xT = resident.tile([P, 16, SQ], f16, tag="xT")
        ck = resident.tile([P, HG, SKV], f16, tag="ck")
        cv = resident.tile([P, HG, 8, D_HEAD], f16, tag="cv")
        cosf = const.tile([P, SQ], f32, tag="cosf")
        sinn = const.tile([P, SQ], f32, tag="sinn")
        ones = const.tile([P, P], f16, tag="ones")
        nc.vector.memset(ones[:], 1.0)

        qkT = resident.tile([P, 16, SQ], f16, tag="qkT")
        v_new = resident.tile([P, 2, 8, 512], f16, tag="v_new")
        attn_T = resident.tile([P, HG, SQ], f16, tag="attn_T")

        BIG = ["sc0", "sc1"]          # 2-bank [128,1024] tags
        SMALL = ["a0", "a1", "d0", "d1"]  # 1-bank [128,512] tags

        # ---- phase 1b: V projection [s, dh] ----
        for c in range(2):
            big = [ptile(BIG[p], f"ps_vb{p}_{c}") for p in range(2)]
            sml = [ptile1(SMALL[p], f"ps_vs{p}_{c}") for p in range(4)]
            accs = [big[0][:, 0:512], big[0][:, 512:1024],
                    big[1][:, 0:512], big[1][:, 512:1024],
                    sml[0][:], sml[1][:], sml[2][:], sml[3][:]]
            for k in range(16):
                if c == 0:
                    nc.sync.dma_start(xT[:, k:k + 1, :], d_xt[:, k:k + 1, :])
                wvt = wst_pool.tile([P, 512], f16, tag="wv")
                # first weight tile races the xT load on the other DMA queue
                if c == 0 and k < 2:
                    nc.gpsimd.dma_start(wvt[:], d_wv[:, k, ts(c, 512)])
                else:
                    nc.sync.dma_start(wvt[:], d_wv[:, k, ts(c, 512)])
                for st in range(8):
                    nc.tensor.matmul(accs[st], xT[:, k, ts(st, P)], wvt[:],
                                     start=(k == 0), stop=(k == 15))
            for st in range(8):
                if st % 2 == 0:
                    nc.scalar.copy(v_new[:, c, st, :], accs[st])
                else:
                    nc.vector.tensor_copy(v_new[:, c, st, :], accs[st])

        nc.sync.dma_start(cosf[:], d_cos[:])
        nc.sync.dma_start(sinn[:], d_sin[:])
        nc.gpsimd.dma_start(ck[:], d_ck[:])
        nc.gpsimd.dma_start(cv[:], d_cv[:])

        # ---- phase 1a: QK^T projection + RoPE ----
        for m in range(16):
            wt = wqk_pool.tile([P, 16, P], f16, tag="wqk")
            nc.sync.dma_start(wt[:], d_wqk[:, :, ts(m, P)])
            if m < 14:
                ps = ptile(BIG[m % 2], f"ps_qk{m}")
                chunks = [ps[:, 0:512], ps[:, 512:1024]]
            else:
                pr = [ptile1(SMALL[2 * (m - 14) + c], f"ps_qk{m}_{c}")
                      for c in range(2)]
                chunks = [pr[0][:], pr[1][:]]
            for c in range(2):
                for k in range(16):
                    nc.tensor.matmul(chunks[c], wt[:, k, :],
                                     xT[:, k, ts(c, 512)],
                                     start=(k == 0), stop=(k == 15))
            # RoPE: out[0:64] = p[0:64]*cos - p[64:]*sin
            #       out[64:]  = p[64:]*cos + p[0:64]*sin
            # (partition-crossing DVE reads straight from PSUM)
            t1 = tmp_pool.tile([P, SQ], f16, tag="t1")
            t0 = tmp_pool.tile([P, SQ], f16, tag="t0")
            for c in range(2):
                pc = chunks[c]
                nc.vector.tensor_tensor(t1[0:64, ts(c, 512)], pc[64:128, :],
                                        sinn[0:64, ts(c, 512)], MUL)
                nc.vector.tensor_tensor(t1[64:128, ts(c, 512)], pc[0:64, :],
                                        sinn[64:128, ts(c, 512)], MUL)
                nc.vector.tensor_tensor(t0[:, ts(c, 512)], pc[:],
                                        cosf[:, ts(c, 512)], MUL)
            nc.vector.tensor_tensor(qkT[:, m, :], t0[:], t1[:], ADD)

        # ---- phase 2: attention, flat (h, tt) pipeline ----
        # slot s = h*16+tt; scores+exp at slot s, av/denom lag DEPTH slots
        DEPTH = 6
        es_q = {}
        ps_av = ps_dn = None

        def vtile(h, tt):
            if tt < 8:
                return cv[:, h, tt, :]
            return v_new[:, h // 4, tt - 8, ts(h % 4, P)]

        def sc_exp(slot):
            h, tt = slot // 16, slot % 16
            kt = (ck[:, h, ts(tt, P)] if tt < 8
                  else qkT[:, 8 + h, ts(tt - 8, P)])
            ps_sc = ptile(BIG[slot % 2], f"ps_sc{h}_{tt}")
            for c in range(2):
                nc.tensor.matmul(ps_sc[:, ts(c, 512)], kt, qkT[:, h, ts(c, 512)],
                                 start=True, stop=True)
            es = exp_pool.tile([P, SQ], f16, tag="es", name=f"es{h}_{tt}")
            nc.scalar.activation(es[:], ps_sc[:], EXP, scale=SCALE)
            es_q[slot] = es

        def av_dn(slot):
            nonlocal ps_av, ps_dn
            h, tt = slot // 16, slot % 16
            if tt == 0:
                ps_av = [ptile1(SMALL[c], f"ps_av{h}_{c}") for c in range(2)]
                ps_dn = [ptile1(SMALL[2 + c], f"ps_dn{h}_{c}") for c in range(2)]
            es = es_q.pop(slot)
            for c in range(2):
                nc.tensor.matmul(ps_av[c][:], vtile(h, tt), es[:, ts(c, 512)],
                                 start=(tt == 0), stop=(tt == 15))
                nc.tensor.matmul(ps_dn[c][:], ones[:], es[:, ts(c, 512)],
                                 start=(tt == 0), stop=(tt == 15))
            if tt == 15:
                for c in range(2):
                    recip = recip_pool.tile([P, 512], f32, tag=f"recip{c}",
                                            name=f"recip{h}_{c}")
                    nc.vector.reciprocal(recip[:], ps_dn[c][:])
                    nc.vector.tensor_tensor(attn_T[:, h, ts(c, 512)],
                                            ps_av[c][:], recip[:], MUL)

        for slot in range(128):
            sc_exp(slot)
            if slot >= DEPTH:
                av_dn(slot - DEPTH)
        for slot in range(128 - DEPTH, 128):
            av_dn(slot)

        # ---- phase 3: output projection ----
        for c4 in range(4):
            big = [ptile(BIG[p], f"ps_ob{p}_{c4}") for p in range(2)]
            sml = [ptile1(SMALL[p], f"ps_os{p}_{c4}") for p in range(4)]
            accs = [big[0][:, 0:512], big[0][:, 512:1024],
                    big[1][:, 0:512], big[1][:, 512:1024],
                    sml[0][:], sml[1][:], sml[2][:], sml[3][:]]
            for h in range(8):
                wot = wst_pool.tile([P, 512], f16, tag="wo")
                nc.sync.dma_start(wot[:], d_wo[:, h, ts(c4, 512)])
                for st in range(8):
                    nc.tensor.matmul(accs[st], attn_T[:, h, ts(st, P)], wot[:],
                                     start=(h == 0), stop=(h == 7))
            for st in range(8):
                og = og_pool.tile([P, 512], f32, tag="og", name=f"og{st}_{c4}")
                if st % 2 == 0:
                    nc.scalar.copy(og[:], accs[st])
                else:
                    nc.vector.tensor_copy(og[:], accs[st])
                if st % 2 == 0:
                    nc.gpsimd.dma_start(d_out[ts(st, P), ts(c4, 512)], og[:])
                else:
                    nc.sync.dma_start(d_out[ts(st, P), ts(c4, 512)], og[:])

    nc.compile()
    return nc


def _get_module():
    if "nc" not in _BUILD_CACHE:
        _BUILD_CACHE["nc"] = build_module()
    return _BUILD_CACHE["nc"]


def _prep_core_inputs(x, cache_k, cache_v, w_qkv, w_o, cosf, sinn, b, g):
    heads = list(range(g * HG, (g + 1) * HG))
    # column indices in w_qkv: head H -> q: 384H..384H+128, k: +128.., v: +256..
    qcols = np.concatenate([np.arange(384 * H, 384 * H + 128) for H in heads])
    kcols = qcols + 128
    vcols = qcols + 256

    xt = np.ascontiguousarray(x[b].T.reshape(16, P, SQ).transpose(1, 0, 2)).astype(F16)
    w_qk = w_qkv[:, np.concatenate([qcols, kcols])]             # [2048, 2048]
    wqk = np.ascontiguousarray(w_qk.reshape(16, P, 2048).transpose(1, 0, 2)).astype(F16)
    w_v = w_qkv[:, vcols]                                       # [2048, 1024]
    wv = np.ascontiguousarray(w_v.reshape(16, P, 1024).transpose(1, 0, 2)).astype(F16)
    ckt = np.ascontiguousarray(cache_k[b, heads].transpose(2, 0, 1)).astype(F16)
    cvt = np.ascontiguousarray(
        cache_v[b, heads].reshape(HG, 8, P, D_HEAD).transpose(2, 0, 1, 3)).astype(F16)
    rows = np.concatenate([np.arange(P * H, P * (H + 1)) for H in heads])
    wo = np.ascontiguousarray(w_o[rows].reshape(HG, P, 2048).transpose(1, 0, 2)).astype(F16)
    return {"xt": xt, "wqk": wqk, "wv": wv, "ck": ckt, "cv": cvt, "wo": wo,
            "cosf": cosf, "sinn": sinn}


def kernel(x, cache_k, cache_v, w_qkv, w_o, trace=False):
    from concourse import bass_utils

    nc = _get_module()
    cosf, sinn = _rope_tables()
    x = np.asarray(x); cache_k = np.asarray(cache_k); cache_v = np.asarray(cache_v)
    w_qkv = np.asarray(w_qkv); w_o = np.asarray(w_o)

    in_maps = []
    for core in range(N_CORES):
        b, g = core // 2, core % 2
        in_maps.append(_prep_core_inputs(x, cache_k, cache_v, w_qkv, w_o,
                                         cosf, sinn, b, g))

    res = bass_utils.run_bass_kernel_spmd(nc, in_maps,
                                          core_ids=list(range(N_CORES)),
                                          trace=trace)
    _BUILD_CACHE["last_result"] = res
    out = np.zeros((B, SQ, D_MODEL), dtype=np.float32)
    for core in range(N_CORES):
        out[core // 2] += res.results[core]["out"]
    return out


if __name__ == "__main__":
    rng = np.random.default_rng(0)
    ins = {
        "x": rng.standard_normal((B, SQ, D_MODEL), dtype=np.float32),
        "cache_k": rng.standard_normal((B, N_HEADS, SKV, D_HEAD), dtype=np.float32),
        "cache_v": rng.standard_normal((B, N_HEADS, SKV, D_HEAD), dtype=np.float32),
        "w_qkv": rng.standard_normal((D_MODEL, 3 * D_MODEL), dtype=np.float32) * D_MODEL ** -0.5,
        "w_o": rng.standard_normal((D_MODEL, D_MODEL), dtype=np.float32) * D_MODEL ** -0.5,
    }
    out = kernel(**ins)
    print("out", out.shape, out.dtype, float(np.abs(out).max()))



# revision 2
# speedup vs baseline: 1.0038x; 1.0038x over previous
"""Tensor-parallel attention kernel for TRN2 (8 NeuronCores), v2.

Sharding: 2D grid — 4 batches x 2 head-groups (8 heads each). Core c handles
batch b = c // 2, head-group g = c % 2.

Key speedups over v1:
  1. All projections (QKV and O) run as fp8 DoubleRow matmuls with an
     e4m3-hi + e5m2-lo 3-term decomposition (X*W ~ Xh*Wh + Xh*Wl + Xl*Wh).
     DoubleRow contracts 256 rows per instruction at 0.5 cycles/row, so the
     3-term group costs 0.75x one f16 matmul while keeping ~0.3% error.
  2. Softmax denominator via DVE elementwise accumulation of exp tiles plus
     one small ones-matmul per head (replaces the per-slot ones-matmul
     ridealong that cost a full extra AV pass on the PE).
  3. QK projection matmuls are interleaved into the attention slot loop so
     the scalar engine (exp) is never the exposed critical path.
  4. f16 output stores; wo streamed ahead per c4 block.

Scales: x is pre-scaled by SX=4, all weights by SW=64 (keeps fp8 hi parts in
the e4m3 normal range). cache_k/cache_v are pre-scaled by SX*SW so scores
carry a uniform (SX*SW)^2 factor folded into the exp scale; the O-projection
eviction divides by SX*SW*SW.
"""

from contextlib import ExitStack

import numpy as np
import ml_dtypes

B, SQ, SKV = 4, 1024, 1024
D_MODEL = 2048
N_HEADS = 16
D_HEAD = 128
ROPE_THETA = 10000.0
N_CORES = 8
HG = 8  # heads per core
P = 128

F16 = np.float16
E4 = ml_dtypes.float8_e4m3
E5 = ml_dtypes.float8_e5m2

SX = 4.0
SW = 64.0
SCL = SX * SW          # scale carried by q, k, v, attn
OUT_SCALE = 1.0 / (SCL * SW)   # applied at O eviction

_BUILD_CACHE = {}


def _rope_tables():
    """cosf/sinn [128, 1024] f16 for new-token positions SKV + s."""
    inv_freq = 1.0 / (ROPE_THETA ** (np.arange(0, D_HEAD, 2, dtype=np.float32) / D_HEAD))
    pos = (SKV + np.arange(SQ, dtype=np.float32))
    ang = pos[:, None] * inv_freq[None, :]           # [S, 64]
    cos = np.cos(ang).astype(np.float32).T           # [64, S]
    sin = np.sin(ang).astype(np.float32).T
    cosf = np.concatenate([cos, cos], axis=0)        # [128, S]
    sinn = np.concatenate([-sin, sin], axis=0)       # [128, S]
    return (np.ascontiguousarray(cosf).astype(F16),
            np.ascontiguousarray(sinn).astype(F16))


def build_module():
    import concourse.mybir as mybir
    import concourse.tile as tile
    from concourse import bacc
    from concourse.bass import ts

    f32 = mybir.dt.float32
    f16 = mybir.dt.float16
    f8e4 = mybir.dt.float8e4
    f8e5 = mybir.dt.float8e5
    DR = mybir.MatmulPerfMode.DoubleRow

    nc = bacc.Bacc("TRN2", target_bir_lowering=False, debug=False,
                   num_devices=N_CORES)

    # fp8 packed tensors use sub-row packing [P, j, 2, cols]:
    # contraction row k = 256*j + 128*i + p.
    d_x4 = nc.dram_tensor("x4", [P, 8, 2, SQ], f8e4, kind="ExternalInput").ap()
    d_x5 = nc.dram_tensor("x5", [P, 8, 2, SQ], f8e5, kind="ExternalInput").ap()
    d_wqk4 = nc.dram_tensor("wqk4", [P, 16, 8, 2, P], f8e4, kind="ExternalInput").ap()
    d_wqk5 = nc.dram_tensor("wqk5", [P, 16, 8, 2, P], f8e5, kind="ExternalInput").ap()
    d_wv4 = nc.dram_tensor("wv4", [P, 8, 2, 1024], f8e4, kind="ExternalInput").ap()
    d_wv5 = nc.dram_tensor("wv5", [P, 8, 2, 1024], f8e5, kind="ExternalInput").ap()
    d_ck = nc.dram_tensor("ck", [P, HG, SKV], f16, kind="ExternalInput").ap()
    d_cv = nc.dram_tensor("cv", [P, HG, 8, D_HEAD], f16, kind="ExternalInput").ap()
    # wo packed for DoubleRow over head pairs: [P(dh), 8 heads, 2048 cols]
    d_wo4 = nc.dram_tensor("wo4", [P, HG, 2048], f8e4, kind="ExternalInput").ap()
    d_wo5 = nc.dram_tensor("wo5", [P, HG, 2048], f8e5, kind="ExternalInput").ap()
    d_cos = nc.dram_tensor("cosf", [P, SQ], f16, kind="ExternalInput").ap()
    d_sin = nc.dram_tensor("sinn", [P, SQ], f16, kind="ExternalInput").ap()
    d_out = nc.dram_tensor("out", [SQ, D_MODEL], f16, kind="ExternalOutput").ap()

    EXP = mybir.ActivationFunctionType.Exp
    COPY = mybir.ActivationFunctionType.Copy
    MUL = mybir.AluOpType.mult
    ADD = mybir.AluOpType.add
    SUB = mybir.AluOpType.subtract
    ESCALE = float(D_HEAD) ** -0.5 / (SCL * SCL)

    with tile.TileContext(nc) as tc, ExitStack() as ctx:
        const = ctx.enter_context(tc.tile_pool(name="const", bufs=1))
        res = ctx.enter_context(tc.tile_pool(name="res", bufs=1))
        wqk_pool = ctx.enter_context(tc.tile_pool(name="wqk", bufs=3))
        wv_pool = ctx.enter_context(tc.tile_pool(name="wv", bufs=3))
        wo_pool = ctx.enter_context(tc.tile_pool(name="wo", bufs=2))
        tmp_pool = ctx.enter_context(tc.tile_pool(name="tmp", bufs=2))
        es_pool = ctx.enter_context(tc.tile_pool(name="es", bufs=6))
        dn_pool = ctx.enter_context(tc.tile_pool(name="dn", bufs=4))
        rc_pool = ctx.enter_context(tc.tile_pool(name="rc", bufs=2))
        og_pool = ctx.enter_context(tc.tile_pool(name="og", bufs=4))
        pp = ctx.enter_context(tc.tile_pool(name="pp", bufs=1, space="PSUM"))

        # ---- resident tiles ----
        x4 = res.tile([P, 8, 2, SQ], f8e4, tag="x4")
        x5 = res.tile([P, 8, 2, SQ], f8e5, tag="x5")
        ck = res.tile([P, HG, SKV], f16, tag="ck")
        cv = res.tile([P, HG, 8, D_HEAD], f16, tag="cv")
        qkT = res.tile([P, 16, SQ], f16, tag="qkT")
        v_new = res.tile([P, 8, 1024], f16, tag="v_new")
        A4 = res.tile([P, HG, SQ], f8e4, tag="A4")
        A5 = res.tile([P, HG, SQ], f8e5, tag="A5")
        cosf = const.tile([P, SQ], f16, tag="cosf")
        sinn = const.tile([P, SQ], f16, tag="sinn")
        ones = const.tile([P, P], f16, tag="ones")
        nc.vector.memset(ones[:], 1.0)

        # PSUM tags: B0/B1 are [128,1024] (2 banks each), S0/S1/A0/A1 are
        # [128,512] (1 bank each). 8 banks total.
        def pbig(tag, name):
            return pp.tile([P, SQ], f32, tag=tag, name=name)

        def psml(tag, name):
            return pp.tile([P, 512], f32, tag=tag, name=name)

        # ---------------- phase V: V projection, fp8 DoubleRow ----------
        # out v_new[q, vcols]; stationary x-chunk [128,2,128q], moving wv
        # [128,2,512vc]. Two half-phases (c = vcol half) of 8 q-chunk groups.
        wv_tiles = {}

        def wv_load(c, j):
            w4t = wv_pool.tile([P, 2, 512], f8e4, tag="wv4")
            w5t = wv_pool.tile([P, 2, 512], f8e5, tag="wv5")
            nc.sync.dma_start(w4t[:], d_wv4[:, j, :, ts(c, 512)])
            nc.sync.dma_start(w5t[:], d_wv5[:, j, :, ts(c, 512)])
            wv_tiles[(c, j)] = (w4t, w5t)

        nc.sync.dma_start(x4[:, 0:1], d_x4[:, 0:1])
        wv_load(0, 0)
        nc.sync.dma_start(x5[:, 0:1], d_x5[:, 0:1])
        nc.sync.dma_start(x4[:, 1:2], d_x4[:, 1:2])
        nc.sync.dma_start(x5[:, 1:2], d_x5[:, 1:2])
        wv_load(0, 1)
        for c in range(2):
            big = [pbig("B0", f"ps_vb0_{c}"), pbig("B1", f"ps_vb1_{c}")]
            sml = [psml(t, f"ps_v{t}_{c}") for t in ("S0", "S1", "A0", "A1")]
            accs = [big[0][:, 0:512], big[0][:, 512:1024],
                    big[1][:, 0:512], big[1][:, 512:1024],
                    sml[0][:], sml[1][:], sml[2][:], sml[3][:]]
            for j in range(8):
                if c == 0 and j < 6:
                    nc.sync.dma_start(x4[:, j + 2:j + 3], d_x4[:, j + 2:j + 3])
                    nc.sync.dma_start(x5[:, j + 2:j + 3], d_x5[:, j + 2:j + 3])
                if j < 6:
                    wv_load(c, j + 2)
                elif c == 0:
                    wv_load(1, j - 6)
                if c == 1:
                    # big resident loads ride the sync queue here so they
                    # sequence behind the latency-critical phase-V stream
                    if j == 0:
                        nc.sync.dma_start(ck[:, 0:4], d_ck[:, 0:4])
                    elif j == 1:
                        nc.sync.dma_start(ck[:, 4:8], d_ck[:, 4:8])
                    elif j == 2:
                        nc.sync.dma_start(cv[:, 0:4], d_cv[:, 0:4])
                    elif j == 3:
                        nc.sync.dma_start(cv[:, 4:8], d_cv[:, 4:8])
                    elif j == 4:
                        nc.sync.dma_start(cosf[:], d_cos[:])
                        nc.sync.dma_start(sinn[:], d_sin[:])
                wv4t, wv5t = wv_tiles.pop((c, j))
                # x4*wv4 terms first so the PE can start before x5/wv5 land
                for st in range(8):
                    nc.tensor.matmul(accs[st], x4[:, j, :, ts(st, P)], wv4t[:],
                                     start=(j == 0), stop=False,
                                     perf_mode=DR)
                for st in range(8):
                    nc.tensor.matmul(accs[st], x4[:, j, :, ts(st, P)], wv5t[:],
                                     start=False, stop=False, perf_mode=DR)
                    nc.tensor.matmul(accs[st], x5[:, j, :, ts(st, P)], wv4t[:],
                                     start=False, stop=(j == 7),
                                     perf_mode=DR)
            for st in range(8):
                if st % 2 == 0:
                    nc.scalar.activation(v_new[:, st, ts(c, 512)], accs[st], COPY)
                else:
                    nc.vector.tensor_copy(v_new[:, st, ts(c, 512)], accs[st])


        # ---------------- QK projection helpers -------------------------
        # m in 0..15: output col-chunk (m<8: q of head m; m>=8: k of head m-8)
        # Each m-block: psum [128,1024] (B0 for q / B1 for k), 48 DoubleRow
        # matmuls (8 j-steps x 2 c-halves x 3 terms).
        def qk_load(m):
            w4t = wqk_pool.tile([P, 8, 2, P], f8e4, tag="wqk4")
            w5t = wqk_pool.tile([P, 8, 2, P], f8e5, tag="wqk5")
            nc.sync.dma_start(w4t[:], d_wqk4[:, m])
            nc.sync.dma_start(w5t[:], d_wqk5[:, m])
            return w4t, w5t

        def qk_mm(psm, w4t, w5t, j, c):
            xm4 = x4[:, j, :, ts(c, 512)]
            xm5 = x5[:, j, :, ts(c, 512)]
            out = psm[:, ts(c, 512)]
            for t, (wt, xt) in enumerate([(w4t[:, j], xm4), (w4t[:, j], xm5),
                                          (w5t[:, j], xm4)]):
                nc.tensor.matmul(out, wt, xt, start=(j == 0 and t == 0),
                                 stop=(j == 7 and t == 2), perf_mode=DR)

        def qk_rope(psm, m):
            # out[0:64] = p[0:64]*cos - p[64:]*sin ; out[64:] = p[64:]*cos + p[0:64]*sin
            t1 = tmp_pool.tile([P, SQ], f16, tag="t1")
            t0 = tmp_pool.tile([P, SQ], f16, tag="t0")
            nc.vector.tensor_tensor(t1[0:64, :], psm[64:128, :], sinn[0:64, :], MUL)
            nc.vector.tensor_tensor(t1[64:128, :], psm[0:64, :], sinn[64:128, :], MUL)
            nc.vector.tensor_tensor(t0[:], psm[:], cosf[:], MUL)
            nc.vector.tensor_tensor(qkT[:, m, :], t0[:], t1[:], ADD)

        # ---------------- attention slot helpers -------------------------
        es_q = {}
        dn_acc = {}

        def sc_exp(h, tt):
            kt = (ck[:, h, ts(tt, P)] if tt < 8
                  else qkT[:, 8 + h, ts(tt - 8, P)])
            ps0 = psml("S0", f"ps_sc0_{h}_{tt}")
            ps1 = psml("S1", f"ps_sc1_{h}_{tt}")
            nc.tensor.matmul(ps0[:], kt, qkT[:, h, 0:512], start=True, stop=True)
            nc.tensor.matmul(ps1[:], kt, qkT[:, h, 512:1024], start=True, stop=True)
            es = es_pool.tile([P, SQ], f16, tag="es", name=f"es{h}_{tt}")
            nc.scalar.activation(es[:, 0:512], ps0[:], EXP, scale=ESCALE)
            nc.scalar.activation(es[:, 512:1024], ps1[:], EXP, scale=ESCALE)
            es_q[(h, tt)] = es
            # dn accumulation on DVE (f16, SBUF-only -> fast mode)
            if tt == 0:
                dn_acc[h] = es
            else:
                nd = dn_pool.tile([P, SQ], f16, tag="dn", name=f"dn{h}_{tt}")
                nc.vector.tensor_tensor(nd[:], dn_acc[h][:], es[:], ADD)
                dn_acc[h] = nd

        def vtile(h, tt):
            if tt < 8:
                return cv[:, h, tt, :]
            return v_new[:, tt - 8, ts(h, P)]

        ps_av = [None, None]

        def av(h, tt):
            if tt == 0:
                ps_av[0] = psml("A0", f"ps_av0_{h}")
                ps_av[1] = psml("A1", f"ps_av1_{h}")
            es = es_q.pop((h, tt))
            for c in range(2):
                nc.tensor.matmul(ps_av[c][:], vtile(h, tt), es[:, ts(c, 512)],
                                 start=(tt == 0), stop=(tt == 15))

        def head_finish(h):
            # denominator broadcast matmul into B1 (free between K-blocks;
            # for the last head B1 is reserved for the O pre-fill, so use the
            # S banks which the sc pipeline no longer needs), then recip +
            # normalize + fp8 hi/lo split of attn.
            dn = dn_acc.pop(h)
            if h == 7:
                halves = [psml("S0", "ps_dn7a")[:], psml("S1", "ps_dn7b")[:]]
            else:
                ps_dn = pbig("B1", f"ps_dn{h}")
                halves = [ps_dn[:, 0:512], ps_dn[:, 512:1024]]
            nc.tensor.matmul(halves[0], ones[:], dn[:, 0:512],
                             start=True, stop=True)
            nc.tensor.matmul(halves[1], ones[:], dn[:, 512:1024],
                             start=True, stop=True)
            recip = rc_pool.tile([P, SQ], f16, tag="recip", name=f"recip{h}")
            with nc.allow_low_precision(reason="softmax recip in f16 is ample"):
                nc.vector.reciprocal(recip[:, 0:512], halves[0])
                nc.vector.reciprocal(recip[:, 512:1024], halves[1])
            t = tmp_pool.tile([P, SQ], f16, tag="attn", name=f"attn{h}")
            for c in range(2):
                nc.vector.tensor_tensor(t[:, ts(c, 512)], ps_av[c][:],
                                        recip[:, ts(c, 512)], MUL)
            if h == 7:
                # epilogue: ACT/DVE are idle and the O phase waits on head 7
                nc.scalar.activation(A4[:, h, :], t[:], COPY)
                nc.vector.tensor_tensor(A5[:, h, :], t[:], A4[:, h, :], SUB)
            else:
                # steady state: ACT (exp) and DVE (dn/rope) are the tight
                # engines; the idle Pool engine absorbs the fp8 split
                nc.gpsimd.tensor_copy(A4[:, h, :], t[:])
                nc.gpsimd.tensor_tensor(A5[:, h, :], t[:], A4[:, h, :], SUB)

        # ---------------- prologue: qk(0) ---------------------------------
        w4t, w5t = qk_load(0)
        psq = pbig("B0", "ps_qk0")
        for j in range(8):
            for c in range(2):
                qk_mm(psq, w4t, w5t, j, c)
        qk_rope(psq, 0)
        w4t, w5t = qk_load(8)
        psk = pbig("B1", "ps_qk8")
        for j in range(8):
            for c in range(2):
                qk_mm(psk, w4t, w5t, j, c)
        qk_rope(psk, 8)

        # ---------------- merged slot loop --------------------------------
        # slot s = 16h + tt. Per slot: sc+exp+dn for (h, tt); 6 qk matmuls of
        # head h+1 (m = h+1 during tt<8, m = 9+h during tt>=8); av lagged 3.
        qk_state = {}

        def qk_piece(h1, tt):
            # head h1's q block over tt=0..7 (6 mm per slot), k block 8..15
            m = h1 if tt < 8 else 8 + h1
            jj = tt % 8  # j index within the block
            if jj == 0:
                qk_state["w"] = qk_load(m)
                qk_state["ps"] = pbig("B0" if tt < 8 else "B1", f"ps_qk{m}")
            w4t, w5t = qk_state["w"]
            for c in range(2):
                qk_mm(qk_state["ps"], w4t, w5t, jj, c)
            if jj == 7:
                qk_rope(qk_state["ps"], m)

        wo_tiles = {}

        def wo_load(c4):
            wo4t = wo_pool.tile([P, HG, 512], f8e4, tag="wo4")
            wo5t = wo_pool.tile([P, HG, 512], f8e5, tag="wo5")
            nc.sync.dma_start(wo4t[:], d_wo4[:, :, ts(c4, 512)])
            nc.sync.dma_start(wo5t[:], d_wo5[:, :, ts(c4, 512)])
            wo_tiles[c4] = (wo4t, wo5t)

        # O-projection pre-fill: head 7's slots carry no qk matmuls, so the
        # PE idles behind ACT's exp stream there. Fill the holes with the
        # heads-0..5 partial terms (hp 0..2) of O c4=0's first two groups.
        o_pre = {}

        def o_pre_mm(k):
            # k in 0..35: (st4, hp, term); st4 0,1 on B0, st4 2,3 on B1
            st4, rem = divmod(k, 9)
            hp, t = divmod(rem, 3)
            if k == 0:
                o_pre["b0"] = pbig("B0", "ps_ob0_pre")
            elif k == 18:
                o_pre["b1"] = pbig("B1", "ps_ob1_pre")
            wo4t, wo5t = wo_tiles[0]
            big = o_pre["b0"] if st4 < 2 else o_pre["b1"]
            acc = big[:, ts(st4 % 2, 512)]
            a4s = A4[:, 2 * hp:2 * hp + 2, ts(st4, P)]
            a5s = A5[:, 2 * hp:2 * hp + 2, ts(st4, P)]
            w4s = wo4t[:, 2 * hp:2 * hp + 2, :]
            w5s = wo5t[:, 2 * hp:2 * hp + 2, :]
            a, w = [(a4s, w4s), (a4s, w5s), (a5s, w4s)][t]
            nc.tensor.matmul(acc, a, w, start=(rem == 0), stop=False,
                             perf_mode=DR)

        for s in range(128):
            h, tt = s // 16, s % 16
            # B0 groups (st4 0,1) fill slots 113-118; B1 groups (st4 2,3)
            # fill slots 119-127 where the PE otherwise idles behind exp
            if 113 <= s <= 118:
                for k in range(3 * (s - 113), 3 * (s - 112)):
                    o_pre_mm(k)
            elif 119 <= s <= 127:
                for k in range(18 + 2 * (s - 119), min(18 + 2 * (s - 118), 36)):
                    o_pre_mm(k)
            # qk before sc: the next sc reuses the S psum banks that exp(s-1)
            # is still draining, so the qk matmuls buy it latency margin
            if h + 1 < 8:
                qk_piece(h + 1, tt)
            sc_exp(h, tt)
            if tt == 3 and h > 0:
                head_finish(h - 1)
            lag = s - 3
            if lag >= 0:
                av(lag // 16, lag % 16)
            if s == 104:
                wo_load(0)
            elif s == 118:
                wo_load(1)
        for s in range(125, 128):
            av(s // 16, s % 16)
        head_finish(7)

        # ---------------- phase O: output projection, fp8 DoubleRow -------
        # out[q, col] = sum_h attn_h.T @ wo_h ; DoubleRow pairs heads
        # (contraction 256 = 2 heads x 128 dh per instruction).
        for c4 in range(4):
            wo4t, wo5t = wo_tiles.pop(c4)
            if c4 + 2 < 4:
                wo_load(c4 + 2)
            for grp in range(2):
                if grp == 0:
                    if c4 == 0:
                        big = [o_pre["b0"], o_pre["b1"]]
                    else:
                        big = [pbig("B0", f"ps_ob0_{c4}"),
                               pbig("B1", f"ps_ob1_{c4}")]
                    accs = [big[0][:, 0:512], big[0][:, 512:1024],
                            big[1][:, 0:512], big[1][:, 512:1024]]
                else:
                    accs = [psml(t, f"ps_o{t}_{c4}")[:]
                            for t in ("S0", "S1", "A0", "A1")]
                for st4 in range(4):
                    st = grp * 4 + st4
                    hp0 = 3 if (c4 == 0 and grp == 0) else 0
                    for hp in range(hp0, 4):
                        a4s = A4[:, 2 * hp:2 * hp + 2, ts(st, P)]
                        a5s = A5[:, 2 * hp:2 * hp + 2, ts(st, P)]
                        w4s = wo4t[:, 2 * hp:2 * hp + 2, :]
                        w5s = wo5t[:, 2 * hp:2 * hp + 2, :]
                        for t, (a, w) in enumerate([(a4s, w4s), (a4s, w5s),
                                                    (a5s, w4s)]):
                            nc.tensor.matmul(accs[st4], a, w,
                                             start=(hp == 0 and t == 0),
                                             stop=(hp == 3 and t == 2),
                                             perf_mode=DR)
                for st4 in range(4):
                    st = grp * 4 + st4
                    og = og_pool.tile([P, 512], f16, tag="og",
                                      name=f"og{st}_{c4}")
                    if st4 % 2 == 0:
                        nc.scalar.activation(og[:], accs[st4], COPY,
                                             scale=OUT_SCALE)
                    else:
                        nc.vector.tensor_scalar_mul(og[:], accs[st4], OUT_SCALE)
                    nc.sync.dma_start(d_out[ts(st, P), ts(c4, 512)], og[:])

    nc.compile()
    return nc


def _get_module():
    if "nc" not in _BUILD_CACHE:
        _BUILD_CACHE["nc"] = build_module()
    return _BUILD_CACHE["nc"]


def _split8(a):
    hi = a.astype(E4)
    lo = (a - hi.astype(np.float32)).astype(E5)
    return hi, lo


def _pack_rows(a):
    """[2048 rows, cols] -> [128, 8, 2, cols]: row k = 256*j + 128*i + p."""
    cols = a.shape[1]
    return np.ascontiguousarray(
        a.reshape(8, 2, P, cols).transpose(2, 0, 1, 3))


def _prep_core_inputs(x, cache_k, cache_v, w_qkv, w_o, cosf, sinn, b, g):
    heads = list(range(g * HG, (g + 1) * HG))
    qcols = np.concatenate([np.arange(384 * H, 384 * H + 128) for H in heads])
    kcols = qcols + 128
    vcols = qcols + 256

    xt = x[b].T * SX                       # [2048 k, 1024 q] scaled
    xh, xl = _split8(xt.astype(np.float32))
    x4 = _pack_rows(xh.astype(np.float32)).astype(E4)
    x5 = _pack_rows(xl.astype(np.float32)).astype(E5)

    w_qk = w_qkv[:, np.concatenate([qcols, kcols])] * SW    # [2048, 2048]
    wh, wl = _split8(w_qk.astype(np.float32))
    # [128, 16 m, 8 j, 2 i, 128 c]
    wqk4 = np.ascontiguousarray(
        _pack_rows(wh.astype(np.float32)).reshape(P, 8, 2, 16, P)
        .transpose(0, 3, 1, 2, 4)).astype(E4)
    wqk5 = np.ascontiguousarray(
        _pack_rows(wl.astype(np.float32)).reshape(P, 8, 2, 16, P)
        .transpose(0, 3, 1, 2, 4)).astype(E5)

    w_v = w_qkv[:, vcols] * SW                              # [2048, 1024]
    wvh, wvl = _split8(w_v.astype(np.float32))
    wv4 = _pack_rows(wvh.astype(np.float32)).astype(E4)
    wv5 = _pack_rows(wvl.astype(np.float32)).astype(E5)

    ckt = np.ascontiguousarray(
        cache_k[b, heads].transpose(2, 0, 1) * SCL).astype(F16)
    cvt = np.ascontiguousarray(
        cache_v[b, heads].reshape(HG, 8, P, D_HEAD).transpose(2, 0, 1, 3)
        * SCL).astype(F16)

    rows = np.concatenate([np.arange(P * H, P * (H + 1)) for H in heads])
    wot = w_o[rows] * SW                                    # [1024, 2048]
    woh, wol = _split8(wot.astype(np.float32))
    # [128 dh, 8 head, 2048]
    wo4 = np.ascontiguousarray(
        woh.astype(np.float32).reshape(HG, P, 2048).transpose(1, 0, 2)).astype(E4)
    wo5 = np.ascontiguousarray(
        wol.astype(np.float32).reshape(HG, P, 2048).transpose(1, 0, 2)).astype(E5)

    return {"x4": x4, "x5": x5, "wqk4": wqk4, "wqk5": wqk5,
            "wv4": wv4, "wv5": wv5, "ck": ckt, "cv": cvt,
            "wo4": wo4, "wo5": wo5, "cosf": cosf, "sinn": sinn}


def kernel(x, cache_k, cache_v, w_qkv, w_o, trace=False):
    from concourse import bass_utils

    nc = _get_module()
    cosf, sinn = _rope_tables()
    x = np.asarray(x); cache_k = np.asarray(cache_k); cache_v = np.asarray(cache_v)
    w_qkv = np.asarray(w_qkv); w_o = np.asarray(w_o)

    in_maps = []
    for core in range(N_CORES):
        b, g = core // 2, core % 2
        in_maps.append(_prep_core_inputs(x, cache_k, cache_v, w_qkv, w_o,
                                         cosf, sinn, b, g))

    res = bass_utils.run_bass_kernel_spmd(nc, in_maps,
                                          core_ids=list(range(N_CORES)),
                                          trace=trace)
    _BUILD_CACHE["last_result"] = res
    out = np.zeros((B, SQ, D_MODEL), dtype=np.float32)
    for core in range(N_CORES):
        out[core // 2] += res.results[core]["out"].astype(np.float32)
    return out


if __name__ == "__main__":
    rng = np.random.default_rng(0)
    ins = {
        "x": rng.standard_normal((B, SQ, D_MODEL), dtype=np.float32),
        "cache_k": rng.standard_normal((B, N_HEADS, SKV, D_HEAD), dtype=np.float32),
        "cache_v": rng.standard_normal((B, N_HEADS, SKV, D_HEAD), dtype=np.float32),
        "w_qkv": rng.standard_normal((D_MODEL, 3 * D_MODEL), dtype=np.float32) * D_MODEL ** -0.5,
        "w_o": rng.standard_normal((D_MODEL, D_MODEL), dtype=np.float32) * D_MODEL ** -0.5,
    }
    out = kernel(**ins)
    print("out", out.shape, out.dtype, float(np.abs(out).max()))


# revision 3
# speedup vs baseline: 1.0153x; 1.0114x over previous
"""Tensor-parallel attention kernel for TRN2 (8 NeuronCores), v2.

Sharding: 2D grid — 4 batches x 2 head-groups (8 heads each). Core c handles
batch b = c // 2, head-group g = c % 2.

Key speedups over v1:
  1. All projections (QKV and O) run as fp8 DoubleRow matmuls with an
     e4m3-hi + e5m2-lo 3-term decomposition (X*W ~ Xh*Wh + Xh*Wl + Xl*Wh).
     DoubleRow contracts 256 rows per instruction at 0.5 cycles/row, so the
     3-term group costs 0.75x one f16 matmul while keeping ~0.3% error.
  2. Softmax denominator via DVE elementwise accumulation of exp tiles plus
     one small ones-matmul per head (replaces the per-slot ones-matmul
     ridealong that cost a full extra AV pass on the PE).
  3. QK projection matmuls are interleaved into the attention slot loop so
     the scalar engine (exp) is never the exposed critical path.
  4. f16 output stores; wo streamed ahead per c4 block.

Scales: x is pre-scaled by SX=4, all weights by SW=64 (keeps fp8 hi parts in
the e4m3 normal range). cache_k/cache_v are pre-scaled by SX*SW so scores
carry a uniform (SX*SW)^2 factor folded into the exp scale; the O-projection
eviction divides by SX*SW*SW.
"""

from contextlib import ExitStack

import numpy as np
import ml_dtypes

B, SQ, SKV = 4, 1024, 1024
D_MODEL = 2048
N_HEADS = 16
D_HEAD = 128
ROPE_THETA = 10000.0
N_CORES = 8
HG = 8  # heads per core
P = 128

F16 = np.float16
E4 = ml_dtypes.float8_e4m3
E5 = ml_dtypes.float8_e5m2

SX = 4.0
SW = 64.0
SCL = SX * SW          # scale carried by q, k, v, attn
OUT_SCALE = 1.0 / (SCL * SW)   # applied at O eviction

_BUILD_CACHE = {}


def _rope_tables():
    """cosf/sinn [128, 1024] f16 for new-token positions SKV + s."""
    inv_freq = 1.0 / (ROPE_THETA ** (np.arange(0, D_HEAD, 2, dtype=np.float32) / D_HEAD))
    pos = (SKV + np.arange(SQ, dtype=np.float32))
    ang = pos[:, None] * inv_freq[None, :]           # [S, 64]
    cos = np.cos(ang).astype(np.float32).T           # [64, S]
    sin = np.sin(ang).astype(np.float32).T
    cosf = np.concatenate([cos, cos], axis=0)        # [128, S]
    sinn = np.concatenate([-sin, sin], axis=0)       # [128, S]
    return (np.ascontiguousarray(cosf).astype(F16),
            np.ascontiguousarray(sinn).astype(F16))


def build_module():
    import concourse.mybir as mybir
    import concourse.tile as tile
    from concourse import bacc
    from concourse.bass import ts

    f32 = mybir.dt.float32
    f16 = mybir.dt.float16
    f8e4 = mybir.dt.float8e4
    f8e5 = mybir.dt.float8e5
    DR = mybir.MatmulPerfMode.DoubleRow

    nc = bacc.Bacc("TRN2", target_bir_lowering=False, debug=False,
                   num_devices=N_CORES)

    # fp8 packed tensors use sub-row packing [P, j, 2, cols]:
    # contraction row k = 256*j + 128*i + p.
    d_x4 = nc.dram_tensor("x4", [P, 8, 2, SQ], f8e4, kind="ExternalInput").ap()
    d_x5 = nc.dram_tensor("x5", [P, 8, 2, SQ], f8e5, kind="ExternalInput").ap()
    d_wqk4 = nc.dram_tensor("wqk4", [P, 16, 8, 2, P], f8e4, kind="ExternalInput").ap()
    d_wqk5 = nc.dram_tensor("wqk5", [P, 16, 8, 2, P], f8e5, kind="ExternalInput").ap()
    d_wv4 = nc.dram_tensor("wv4", [P, 8, 2, 1024], f8e4, kind="ExternalInput").ap()
    d_wv5 = nc.dram_tensor("wv5", [P, 8, 2, 1024], f8e5, kind="ExternalInput").ap()
    d_ck = nc.dram_tensor("ck", [P, HG, SKV], f16, kind="ExternalInput").ap()
    d_cv = nc.dram_tensor("cv", [P, HG, 8, D_HEAD], f16, kind="ExternalInput").ap()
    # wo packed for DoubleRow over head pairs: [P(dh), 8 heads, 2048 cols]
    d_wo4 = nc.dram_tensor("wo4", [P, HG, 2048], f8e4, kind="ExternalInput").ap()
    d_wo5 = nc.dram_tensor("wo5", [P, HG, 2048], f8e5, kind="ExternalInput").ap()
    d_cos = nc.dram_tensor("cosf", [P, SQ], f16, kind="ExternalInput").ap()
    d_sin = nc.dram_tensor("sinn", [P, SQ], f16, kind="ExternalInput").ap()
    d_out = nc.dram_tensor("out", [SQ, D_MODEL], f16, kind="ExternalOutput").ap()

    EXP = mybir.ActivationFunctionType.Exp
    COPY = mybir.ActivationFunctionType.Copy
    MUL = mybir.AluOpType.mult
    ADD = mybir.AluOpType.add
    SUB = mybir.AluOpType.subtract
    ESCALE = float(D_HEAD) ** -0.5 / (SCL * SCL)

    with tile.TileContext(nc) as tc, ExitStack() as ctx:
        const = ctx.enter_context(tc.tile_pool(name="const", bufs=1))
        res = ctx.enter_context(tc.tile_pool(name="res", bufs=1))
        wqk_pool = ctx.enter_context(tc.tile_pool(name="wqk", bufs=3))
        wv_pool = ctx.enter_context(tc.tile_pool(name="wv", bufs=3))
        wo_pool = ctx.enter_context(tc.tile_pool(name="wo", bufs=2))
        tmp_pool = ctx.enter_context(tc.tile_pool(name="tmp", bufs=2))
        es_pool = ctx.enter_context(tc.tile_pool(name="es", bufs=6))
        dn_pool = ctx.enter_context(tc.tile_pool(name="dn", bufs=4))
        rc_pool = ctx.enter_context(tc.tile_pool(name="rc", bufs=2))
        og_pool = ctx.enter_context(tc.tile_pool(name="og", bufs=4))
        pp = ctx.enter_context(tc.tile_pool(name="pp", bufs=1, space="PSUM"))

        # ---- resident tiles ----
        x4 = res.tile([P, 8, 2, SQ], f8e4, tag="x4")
        x5 = res.tile([P, 8, 2, SQ], f8e5, tag="x5")
        ck = res.tile([P, HG, SKV], f16, tag="ck")
        cv = res.tile([P, HG, 8, D_HEAD], f16, tag="cv")
        qkT = res.tile([P, 16, SQ], f16, tag="qkT")
        v_new = res.tile([P, 8, 1024], f16, tag="v_new")
        A4 = res.tile([P, HG, SQ], f8e4, tag="A4")
        A5 = res.tile([P, HG, SQ], f8e5, tag="A5")
        cosf = const.tile([P, SQ], f16, tag="cosf")
        sinn = const.tile([P, SQ], f16, tag="sinn")
        ones = const.tile([P, P], f16, tag="ones")
        nc.vector.memset(ones[:], 1.0)

        # PSUM tags: B0/B1 are [128,1024] (2 banks each), S0/S1/A0/A1 are
        # [128,512] (1 bank each). 8 banks total.
        def pbig(tag, name):
            return pp.tile([P, SQ], f32, tag=tag, name=name)

        def psml(tag, name):
            return pp.tile([P, 512], f32, tag=tag, name=name)

        # ---------------- phase V: V projection, fp8 DoubleRow ----------
        # out v_new[q, vcols]; stationary x-chunk [128,2,128q], moving wv
        # [128,2,512vc]. Two half-phases (c = vcol half) of 8 q-chunk groups.
        wv_tiles = {}

        def wv_load(c, j):
            w4t = wv_pool.tile([P, 2, 512], f8e4, tag="wv4")
            w5t = wv_pool.tile([P, 2, 512], f8e5, tag="wv5")
            nc.sync.dma_start(w4t[:], d_wv4[:, j, :, ts(c, 512)])
            nc.sync.dma_start(w5t[:], d_wv5[:, j, :, ts(c, 512)])
            wv_tiles[(c, j)] = (w4t, w5t)

        # first bite: small slice covering the first stationary chunks so the
        # PE's first matmul starts as early as possible
        nc.sync.dma_start(x4[:, 0:1, :, 0:256], d_x4[:, 0:1, :, 0:256])
        wv_load(0, 0)
        nc.sync.dma_start(x4[:, 0:1, :, 256:1024], d_x4[:, 0:1, :, 256:1024])
        nc.sync.dma_start(x5[:, 0:1], d_x5[:, 0:1])
        nc.sync.dma_start(x4[:, 1:2], d_x4[:, 1:2])
        nc.sync.dma_start(x5[:, 1:2], d_x5[:, 1:2])
        wv_load(0, 1)
        for c in range(2):
            big = [pbig("B0", f"ps_vb0_{c}"), pbig("B1", f"ps_vb1_{c}")]
            sml = [psml(t, f"ps_v{t}_{c}") for t in ("S0", "S1", "A0", "A1")]
            accs = [big[0][:, 0:512], big[0][:, 512:1024],
                    big[1][:, 0:512], big[1][:, 512:1024],
                    sml[0][:], sml[1][:], sml[2][:], sml[3][:]]
            for j in range(8):
                if c == 0 and j < 6:
                    nc.sync.dma_start(x4[:, j + 2:j + 3], d_x4[:, j + 2:j + 3])
                    nc.sync.dma_start(x5[:, j + 2:j + 3], d_x5[:, j + 2:j + 3])
                if j < 6:
                    wv_load(c, j + 2)
                elif c == 0:
                    wv_load(1, j - 6)
                if c == 1:
                    # big resident loads ride the sync queue here so they
                    # sequence behind the latency-critical phase-V stream
                    if j == 0:
                        nc.sync.dma_start(ck[:, 0:4], d_ck[:, 0:4])
                    elif j == 1:
                        nc.sync.dma_start(ck[:, 4:8], d_ck[:, 4:8])
                    elif j == 2:
                        nc.sync.dma_start(cv[:, 0:4], d_cv[:, 0:4])
                    elif j == 3:
                        nc.sync.dma_start(cv[:, 4:8], d_cv[:, 4:8])
                    elif j == 4:
                        nc.sync.dma_start(cosf[:], d_cos[:])
                        nc.sync.dma_start(sinn[:], d_sin[:])
                wv4t, wv5t = wv_tiles.pop((c, j))
                # x4*wv4 terms first so the PE can start before x5/wv5 land
                for st in range(8):
                    nc.tensor.matmul(accs[st], x4[:, j, :, ts(st, P)], wv4t[:],
                                     start=(j == 0), stop=False,
                                     perf_mode=DR)
                for st in range(8):
                    nc.tensor.matmul(accs[st], x4[:, j, :, ts(st, P)], wv5t[:],
                                     start=False, stop=False, perf_mode=DR)
                    nc.tensor.matmul(accs[st], x5[:, j, :, ts(st, P)], wv4t[:],
                                     start=False, stop=(j == 7),
                                     perf_mode=DR)
            for st in range(8):
                if st % 2 == 0:
                    nc.scalar.activation(v_new[:, st, ts(c, 512)], accs[st], COPY)
                else:
                    nc.vector.tensor_copy(v_new[:, st, ts(c, 512)], accs[st])


        # ---------------- QK projection helpers -------------------------
        # m in 0..15: output col-chunk (m<8: q of head m; m>=8: k of head m-8)
        # Each m-block: psum [128,1024] (B0 for q / B1 for k), 48 DoubleRow
        # matmuls (8 j-steps x 2 c-halves x 3 terms).
        def qk_load(m):
            w4t = wqk_pool.tile([P, 8, 2, P], f8e4, tag="wqk4")
            w5t = wqk_pool.tile([P, 8, 2, P], f8e5, tag="wqk5")
            nc.sync.dma_start(w4t[:], d_wqk4[:, m])
            nc.sync.dma_start(w5t[:], d_wqk5[:, m])
            return w4t, w5t

        def qk_mm(psm, w4t, w5t, j, c):
            xm4 = x4[:, j, :, ts(c, 512)]
            xm5 = x5[:, j, :, ts(c, 512)]
            out = psm[:, ts(c, 512)]
            for t, (wt, xt) in enumerate([(w4t[:, j], xm4), (w4t[:, j], xm5),
                                          (w5t[:, j], xm4)]):
                nc.tensor.matmul(out, wt, xt, start=(j == 0 and t == 0),
                                 stop=(j == 7 and t == 2), perf_mode=DR)

        def qk_rope(psm, m):
            # out[0:64] = p[0:64]*cos - p[64:]*sin ; out[64:] = p[64:]*cos + p[0:64]*sin
            t1 = tmp_pool.tile([P, SQ], f16, tag="t1")
            t0 = tmp_pool.tile([P, SQ], f16, tag="t0")
            nc.vector.tensor_tensor(t1[0:64, :], psm[64:128, :], sinn[0:64, :], MUL)
            nc.vector.tensor_tensor(t1[64:128, :], psm[0:64, :], sinn[64:128, :], MUL)
            nc.vector.tensor_tensor(t0[:], psm[:], cosf[:], MUL)
            nc.vector.tensor_tensor(qkT[:, m, :], t0[:], t1[:], ADD)

        # ---------------- attention slot helpers -------------------------
        es_q = {}
        dn_acc = {}

        def sc_exp(h, tt):
            kt = (ck[:, h, ts(tt, P)] if tt < 8
                  else qkT[:, 8 + h, ts(tt - 8, P)])
            ps0 = psml("S0", f"ps_sc0_{h}_{tt}")
            ps1 = psml("S1", f"ps_sc1_{h}_{tt}")
            nc.tensor.matmul(ps0[:], kt, qkT[:, h, 0:512], start=True, stop=True)
            nc.tensor.matmul(ps1[:], kt, qkT[:, h, 512:1024], start=True, stop=True)
            es = es_pool.tile([P, SQ], f16, tag="es", name=f"es{h}_{tt}")
            nc.scalar.activation(es[:, 0:512], ps0[:], EXP, scale=ESCALE)
            nc.scalar.activation(es[:, 512:1024], ps1[:], EXP, scale=ESCALE)
            es_q[(h, tt)] = es
            # dn accumulation on DVE (f16, SBUF-only -> fast mode)
            if tt == 0:
                dn_acc[h] = es
            else:
                nd = dn_pool.tile([P, SQ], f16, tag="dn", name=f"dn{h}_{tt}")
                nc.vector.tensor_tensor(nd[:], dn_acc[h][:], es[:], ADD)
                dn_acc[h] = nd

        def vtile(h, tt):
            if tt < 8:
                return cv[:, h, tt, :]
            return v_new[:, tt - 8, ts(h, P)]

        ps_av = [None, None]

        def av(h, tt):
            if tt == 0:
                ps_av[0] = psml("A0", f"ps_av0_{h}")
                ps_av[1] = psml("A1", f"ps_av1_{h}")
            es = es_q.pop((h, tt))
            for c in range(2):
                nc.tensor.matmul(ps_av[c][:], vtile(h, tt), es[:, ts(c, 512)],
                                 start=(tt == 0), stop=(tt == 15))

        def head_finish(h):
            # denominator broadcast matmul into B1 (free between K-blocks;
            # for the last head B1 is reserved for the O pre-fill, so use the
            # S banks which the sc pipeline no longer needs), then recip +
            # normalize + fp8 hi/lo split of attn.
            dn = dn_acc.pop(h)
            recip = rc_pool.tile([P, SQ], f16, tag="recip", name=f"recip{h}")
            t = tmp_pool.tile([P, SQ], f16, tag="attn", name=f"attn{h}")
            if h == 7:
                # epilogue: pipeline by column halves — the O phase's first
                # groups (q cols 0..511) only need the c=0 half of A4/A5, so
                # racing it through cuts the serial tail. ACT/DVE are idle.
                for c in range(2):
                    half = psml(("S0", "S1")[c], f"ps_dn7{c}")[:]
                    nc.tensor.matmul(half, ones[:], dn[:, ts(c, 512)],
                                     start=True, stop=True)
                    with nc.allow_low_precision(reason="f16 recip is ample"):
                        nc.vector.reciprocal(recip[:, ts(c, 512)], half)
                    nc.vector.tensor_tensor(t[:, ts(c, 512)], ps_av[c][:],
                                            recip[:, ts(c, 512)], MUL)
                    nc.scalar.activation(A4[:, h, ts(c, 512)],
                                         t[:, ts(c, 512)], COPY)
                    nc.vector.tensor_tensor(A5[:, h, ts(c, 512)],
                                            t[:, ts(c, 512)],
                                            A4[:, h, ts(c, 512)], SUB)
                return
            ps_dn = pbig("B1", f"ps_dn{h}")
            nc.tensor.matmul(ps_dn[:, 0:512], ones[:], dn[:, 0:512],
                             start=True, stop=True)
            nc.tensor.matmul(ps_dn[:, 512:1024], ones[:], dn[:, 512:1024],
                             start=True, stop=True)
            with nc.allow_low_precision(reason="softmax recip in f16 is ample"):
                nc.vector.reciprocal(recip[:, 0:512], ps_dn[:, 0:512])
                nc.vector.reciprocal(recip[:, 512:1024], ps_dn[:, 512:1024])
            for c in range(2):
                nc.vector.tensor_tensor(t[:, ts(c, 512)], ps_av[c][:],
                                        recip[:, ts(c, 512)], MUL)
            # steady state: ACT (exp) and DVE (dn/rope) are the tight
            # engines; the idle Pool engine absorbs the fp8 split
            nc.gpsimd.tensor_copy(A4[:, h, :], t[:])
            nc.gpsimd.tensor_tensor(A5[:, h, :], t[:], A4[:, h, :], SUB)

        # ---------------- prologue: qk(0) ---------------------------------
        w4t, w5t = qk_load(0)
        psq = pbig("B0", "ps_qk0")
        for j in range(8):
            for c in range(2):
                qk_mm(psq, w4t, w5t, j, c)
        qk_rope(psq, 0)
        w4t, w5t = qk_load(8)
        psk = pbig("B1", "ps_qk8")
        for j in range(8):
            for c in range(2):
                qk_mm(psk, w4t, w5t, j, c)
        qk_rope(psk, 8)

        # ---------------- merged slot loop --------------------------------
        # slot s = 16h + tt. Per slot: sc+exp+dn for (h, tt); 6 qk matmuls of
        # head h+1 (m = h+1 during tt<8, m = 9+h during tt>=8); av lagged 3.
        qk_state = {}

        def qk_piece(h1, tt):
            # head h1's q block over tt=0..7 (6 mm per slot), k block 8..15
            m = h1 if tt < 8 else 8 + h1
            jj = tt % 8  # j index within the block
            if jj == 0:
                qk_state["w"] = qk_load(m)
                qk_state["ps"] = pbig("B0" if tt < 8 else "B1", f"ps_qk{m}")
            w4t, w5t = qk_state["w"]
            for c in range(2):
                qk_mm(qk_state["ps"], w4t, w5t, jj, c)
            if jj == 7:
                qk_rope(qk_state["ps"], m)

        wo_tiles = {}

        def wo_load(c4):
            wo4t = wo_pool.tile([P, HG, 512], f8e4, tag="wo4")
            wo5t = wo_pool.tile([P, HG, 512], f8e5, tag="wo5")
            nc.sync.dma_start(wo4t[:], d_wo4[:, :, ts(c4, 512)])
            nc.sync.dma_start(wo5t[:], d_wo5[:, :, ts(c4, 512)])
            wo_tiles[c4] = (wo4t, wo5t)

        # O-projection pre-fill: head 7's slots carry no qk matmuls, so the
        # PE idles behind ACT's exp stream there. Fill the holes with the
        # heads-0..5 partial terms (hp 0..2) of O c4=0's first two groups.
        o_pre = {}

        def o_pre_mm(k):
            # k in 0..35: (st4, hp, term); st4 0,1 on B0, st4 2,3 on B1
            st4, rem = divmod(k, 9)
            hp, t = divmod(rem, 3)
            if k == 0:
                o_pre["b0"] = pbig("B0", "ps_ob0_pre")
            elif k == 18:
                o_pre["b1"] = pbig("B1", "ps_ob1_pre")
            wo4t, wo5t = wo_tiles[0]
            big = o_pre["b0"] if st4 < 2 else o_pre["b1"]
            acc = big[:, ts(st4 % 2, 512)]
            a4s = A4[:, 2 * hp:2 * hp + 2, ts(st4, P)]
            a5s = A5[:, 2 * hp:2 * hp + 2, ts(st4, P)]
            w4s = wo4t[:, 2 * hp:2 * hp + 2, :]
            w5s = wo5t[:, 2 * hp:2 * hp + 2, :]
            a, w = [(a4s, w4s), (a4s, w5s), (a5s, w4s)][t]
            nc.tensor.matmul(acc, a, w, start=(rem == 0), stop=False,
                             perf_mode=DR)

        for s in range(128):
            h, tt = s // 16, s % 16
            # B0 groups (st4 0,1) fill slots 113-118; B1 groups (st4 2,3)
            # fill slots 119-127 where the PE otherwise idles behind exp
            if 113 <= s <= 118:
                for k in range(3 * (s - 113), 3 * (s - 112)):
                    o_pre_mm(k)
            elif 119 <= s <= 127:
                for k in range(18 + 2 * (s - 119), min(18 + 2 * (s - 118), 36)):
                    o_pre_mm(k)
            # qk before sc: the next sc reuses the S psum banks that exp(s-1)
            # is still draining, so the qk matmuls buy it latency margin
            if h + 1 < 8:
                qk_piece(h + 1, tt)
            sc_exp(h, tt)
            if tt == 3 and h > 0:
                head_finish(h - 1)
            lag = s - 3
            if lag >= 0:
                av(lag // 16, lag % 16)
            if s == 104:
                wo_load(0)
            elif s == 118:
                wo_load(1)
        for s in range(125, 128):
            av(s // 16, s % 16)
        head_finish(7)

        # ---------------- phase O: output projection, fp8 DoubleRow -------
        # out[q, col] = sum_h attn_h.T @ wo_h ; DoubleRow pairs heads
        # (contraction 256 = 2 heads x 128 dh per instruction).
        for c4 in range(4):
            wo4t, wo5t = wo_tiles.pop(c4)
            if c4 + 2 < 4:
                wo_load(c4 + 2)
            for grp in range(2):
                if grp == 0:
                    if c4 == 0:
                        big = [o_pre["b0"], o_pre["b1"]]
                    else:
                        big = [pbig("B0", f"ps_ob0_{c4}"),
                               pbig("B1", f"ps_ob1_{c4}")]
                    accs = [big[0][:, 0:512], big[0][:, 512:1024],
                            big[1][:, 0:512], big[1][:, 512:1024]]
                else:
                    accs = [psml(t, f"ps_o{t}_{c4}")[:]
                            for t in ("S0", "S1", "A0", "A1")]
                for st4 in range(4):
                    st = grp * 4 + st4
                    hp0 = 3 if (c4 == 0 and grp == 0) else 0
                    for hp in range(hp0, 4):
                        a4s = A4[:, 2 * hp:2 * hp + 2, ts(st, P)]
                        a5s = A5[:, 2 * hp:2 * hp + 2, ts(st, P)]
                        w4s = wo4t[:, 2 * hp:2 * hp + 2, :]
                        w5s = wo5t[:, 2 * hp:2 * hp + 2, :]
                        for t, (a, w) in enumerate([(a4s, w4s), (a4s, w5s),
                                                    (a5s, w4s)]):
                            nc.tensor.matmul(accs[st4], a, w,
                                             start=(hp == 0 and t == 0),
                                             stop=(hp == 3 and t == 2),
                                             perf_mode=DR)
                last = (c4 == 3 and grp == 1)
                for st4 in range(4):
                    st = grp * 4 + st4
                    og = og_pool.tile([P, 512], f16, tag="og",
                                      name=f"og{st}_{c4}")
                    if last and st4 == 3:
                        # final chunk: evict+store in column halves on both
                        # engines to shorten the end-of-kernel drain chain
                        nc.scalar.activation(og[:, 0:256], accs[st4][:, 0:256],
                                             COPY, scale=OUT_SCALE)
                        nc.sync.dma_start(
                            d_out[ts(st, P), c4 * 512:c4 * 512 + 256],
                            og[:, 0:256])
                        nc.vector.tensor_scalar_mul(og[:, 256:512],
                                                    accs[st4][:, 256:512],
                                                    OUT_SCALE)
                        nc.sync.dma_start(
                            d_out[ts(st, P), c4 * 512 + 256:c4 * 512 + 512],
                            og[:, 256:512])
                        continue
                    if st4 % 2 == 0:
                        nc.scalar.activation(og[:], accs[st4], COPY,
                                             scale=OUT_SCALE)
                    else:
                        nc.vector.tensor_scalar_mul(og[:], accs[st4], OUT_SCALE)
                    nc.sync.dma_start(d_out[ts(st, P), ts(c4, 512)], og[:])

    nc.compile()
    return nc


def _get_module():
    if "nc" not in _BUILD_CACHE:
        _BUILD_CACHE["nc"] = build_module()
    return _BUILD_CACHE["nc"]


def _split8(a):
    hi = a.astype(E4)
    lo = (a - hi.astype(np.float32)).astype(E5)
    return hi, lo


def _pack_rows(a):
    """[2048 rows, cols] -> [128, 8, 2, cols]: row k = 256*j + 128*i + p."""
    cols = a.shape[1]
    return np.ascontiguousarray(
        a.reshape(8, 2, P, cols).transpose(2, 0, 1, 3))


def _prep_core_inputs(x, cache_k, cache_v, w_qkv, w_o, cosf, sinn, b, g):
    heads = list(range(g * HG, (g + 1) * HG))
    qcols = np.concatenate([np.arange(384 * H, 384 * H + 128) for H in heads])
    kcols = qcols + 128
    vcols = qcols + 256

    xt = x[b].T * SX                       # [2048 k, 1024 q] scaled
    xh, xl = _split8(xt.astype(np.float32))
    x4 = _pack_rows(xh.astype(np.float32)).astype(E4)
    x5 = _pack_rows(xl.astype(np.float32)).astype(E5)

    w_qk = w_qkv[:, np.concatenate([qcols, kcols])] * SW    # [2048, 2048]
    wh, wl = _split8(w_qk.astype(np.float32))
    # [128, 16 m, 8 j, 2 i, 128 c]
    wqk4 = np.ascontiguousarray(
        _pack_rows(wh.astype(np.float32)).reshape(P, 8, 2, 16, P)
        .transpose(0, 3, 1, 2, 4)).astype(E4)
    wqk5 = np.ascontiguousarray(
        _pack_rows(wl.astype(np.float32)).reshape(P, 8, 2, 16, P)
        .transpose(0, 3, 1, 2, 4)).astype(E5)

    w_v = w_qkv[:, vcols] * SW                              # [2048, 1024]
    wvh, wvl = _split8(w_v.astype(np.float32))
    wv4 = _pack_rows(wvh.astype(np.float32)).astype(E4)
    wv5 = _pack_rows(wvl.astype(np.float32)).astype(E5)

    ckt = np.ascontiguousarray(
        cache_k[b, heads].transpose(2, 0, 1) * SCL).astype(F16)
    cvt = np.ascontiguousarray(
        cache_v[b, heads].reshape(HG, 8, P, D_HEAD).transpose(2, 0, 1, 3)
        * SCL).astype(F16)

    rows = np.concatenate([np.arange(P * H, P * (H + 1)) for H in heads])
    wot = w_o[rows] * SW                                    # [1024, 2048]
    woh, wol = _split8(wot.astype(np.float32))
    # [128 dh, 8 head, 2048]
    wo4 = np.ascontiguousarray(
        woh.astype(np.float32).reshape(HG, P, 2048).transpose(1, 0, 2)).astype(E4)
    wo5 = np.ascontiguousarray(
        wol.astype(np.float32).reshape(HG, P, 2048).transpose(1, 0, 2)).astype(E5)

    return {"x4": x4, "x5": x5, "wqk4": wqk4, "wqk5": wqk5,
            "wv4": wv4, "wv5": wv5, "ck": ckt, "cv": cvt,
            "wo4": wo4, "wo5": wo5, "cosf": cosf, "sinn": sinn}


def kernel(x, cache_k, cache_v, w_qkv, w_o, trace=False):
    from concourse import bass_utils

    nc = _get_module()
    cosf, sinn = _rope_tables()
    x = np.asarray(x); cache_k = np.asarray(cache_k); cache_v = np.asarray(cache_v)
    w_qkv = np.asarray(w_qkv); w_o = np.asarray(w_o)

    in_maps = []
    for core in range(N_CORES):
        b, g = core // 2, core % 2
        in_maps.append(_prep_core_inputs(x, cache_k, cache_v, w_qkv, w_o,
                                         cosf, sinn, b, g))

    res = bass_utils.run_bass_kernel_spmd(nc, in_maps,
                                          core_ids=list(range(N_CORES)),
                                          trace=trace)
    _BUILD_CACHE["last_result"] = res
    out = np.zeros((B, SQ, D_MODEL), dtype=np.float32)
    for core in range(N_CORES):
        out[core // 2] += res.results[core]["out"].astype(np.float32)
    return out


if __name__ == "__main__":
    rng = np.random.default_rng(0)
    ins = {
        "x": rng.standard_normal((B, SQ, D_MODEL), dtype=np.float32),
        "cache_k": rng.standard_normal((B, N_HEADS, SKV, D_HEAD), dtype=np.float32),
        "cache_v": rng.standard_normal((B, N_HEADS, SKV, D_HEAD), dtype=np.float32),
        "w_qkv": rng.standard_normal((D_MODEL, 3 * D_MODEL), dtype=np.float32) * D_MODEL ** -0.5,
        "w_o": rng.standard_normal((D_MODEL, D_MODEL), dtype=np.float32) * D_MODEL ** -0.5,
    }
    out = kernel(**ins)
    print("out", out.shape, out.dtype, float(np.abs(out).max()))


# revision 4
# speedup vs baseline: 1.0163x; 1.0010x over previous
"""Tensor-parallel attention kernel for TRN2 (8 NeuronCores), v2.

Sharding: 2D grid — 4 batches x 2 head-groups (8 heads each). Core c handles
batch b = c // 2, head-group g = c % 2.

Key speedups over v1:
  1. All projections (QKV and O) run as fp8 DoubleRow matmuls with an
     e4m3-hi + e5m2-lo 3-term decomposition (X*W ~ Xh*Wh + Xh*Wl + Xl*Wh).
     DoubleRow contracts 256 rows per instruction at 0.5 cycles/row, so the
     3-term group costs 0.75x one f16 matmul while keeping ~0.3% error.
  2. Softmax denominator via DVE elementwise accumulation of exp tiles plus
     one small ones-matmul per head (replaces the per-slot ones-matmul
     ridealong that cost a full extra AV pass on the PE).
  3. QK projection matmuls are interleaved into the attention slot loop so
     the scalar engine (exp) is never the exposed critical path.
  4. f16 output stores; wo streamed ahead per c4 block.

Scales: x is pre-scaled by SX=4, all weights by SW=64 (keeps fp8 hi parts in
the e4m3 normal range). cache_k/cache_v are pre-scaled by SX*SW so scores
carry a uniform (SX*SW)^2 factor folded into the exp scale; the O-projection
eviction divides by SX*SW*SW.
"""

from contextlib import ExitStack

import numpy as np
import ml_dtypes

B, SQ, SKV = 4, 1024, 1024
D_MODEL = 2048
N_HEADS = 16
D_HEAD = 128
ROPE_THETA = 10000.0
N_CORES = 8
HG = 8  # heads per core
P = 128

F16 = np.float16
E4 = ml_dtypes.float8_e4m3
E5 = ml_dtypes.float8_e5m2

SX = 4.0
SW = 64.0
SCL = SX * SW          # scale carried by q, k, v, attn
OUT_SCALE = 1.0 / (SCL * SW)   # applied at O eviction

_BUILD_CACHE = {}


def _rope_tables():
    """cosf/sinn [128, 1024] f16 for new-token positions SKV + s."""
    inv_freq = 1.0 / (ROPE_THETA ** (np.arange(0, D_HEAD, 2, dtype=np.float32) / D_HEAD))
    pos = (SKV + np.arange(SQ, dtype=np.float32))
    ang = pos[:, None] * inv_freq[None, :]           # [S, 64]
    cos = np.cos(ang).astype(np.float32).T           # [64, S]
    sin = np.sin(ang).astype(np.float32).T
    cosf = np.concatenate([cos, cos], axis=0)        # [128, S]
    sinn = np.concatenate([-sin, sin], axis=0)       # [128, S]
    return (np.ascontiguousarray(cosf).astype(F16),
            np.ascontiguousarray(sinn).astype(F16))


def build_module():
    import concourse.mybir as mybir
    import concourse.tile as tile
    from concourse import bacc
    from concourse.bass import ts

    f32 = mybir.dt.float32
    f16 = mybir.dt.float16
    f8e4 = mybir.dt.float8e4
    f8e5 = mybir.dt.float8e5
    DR = mybir.MatmulPerfMode.DoubleRow

    nc = bacc.Bacc("TRN2", target_bir_lowering=False, debug=False,
                   num_devices=N_CORES)

    # fp8 packed tensors use sub-row packing [P, j, 2, cols]:
    # contraction row k = 256*j + 128*i + p.
    d_x4 = nc.dram_tensor("x4", [P, 8, 2, SQ], f8e4, kind="ExternalInput").ap()
    d_x5 = nc.dram_tensor("x5", [P, 8, 2, SQ], f8e5, kind="ExternalInput").ap()
    d_wqk4 = nc.dram_tensor("wqk4", [P, 16, 8, 2, P], f8e4, kind="ExternalInput").ap()
    d_wqk5 = nc.dram_tensor("wqk5", [P, 16, 8, 2, P], f8e5, kind="ExternalInput").ap()
    d_wv4 = nc.dram_tensor("wv4", [P, 8, 2, 1024], f8e4, kind="ExternalInput").ap()
    d_wv5 = nc.dram_tensor("wv5", [P, 8, 2, 1024], f8e5, kind="ExternalInput").ap()
    d_ck = nc.dram_tensor("ck", [P, HG, SKV], f16, kind="ExternalInput").ap()
    d_cv = nc.dram_tensor("cv", [P, HG, 8, D_HEAD], f16, kind="ExternalInput").ap()
    # wo packed for DoubleRow over head pairs: [P(dh), 8 heads, 2048 cols]
    d_wo4 = nc.dram_tensor("wo4", [P, HG, 2048], f8e4, kind="ExternalInput").ap()
    d_wo5 = nc.dram_tensor("wo5", [P, HG, 2048], f8e5, kind="ExternalInput").ap()
    d_cos = nc.dram_tensor("cosf", [P, SQ], f16, kind="ExternalInput").ap()
    d_sin = nc.dram_tensor("sinn", [P, SQ], f16, kind="ExternalInput").ap()
    d_out = nc.dram_tensor("out", [SQ, D_MODEL], f16, kind="ExternalOutput").ap()

    EXP = mybir.ActivationFunctionType.Exp
    COPY = mybir.ActivationFunctionType.Copy
    MUL = mybir.AluOpType.mult
    ADD = mybir.AluOpType.add
    SUB = mybir.AluOpType.subtract
    ESCALE = float(D_HEAD) ** -0.5 / (SCL * SCL)

    with tile.TileContext(nc) as tc, ExitStack() as ctx:
        const = ctx.enter_context(tc.tile_pool(name="const", bufs=1))
        res = ctx.enter_context(tc.tile_pool(name="res", bufs=1))
        wqk_pool = ctx.enter_context(tc.tile_pool(name="wqk", bufs=3))
        wv_pool = ctx.enter_context(tc.tile_pool(name="wv", bufs=3))
        wo_pool = ctx.enter_context(tc.tile_pool(name="wo", bufs=2))
        tmp_pool = ctx.enter_context(tc.tile_pool(name="tmp", bufs=2))
        es_pool = ctx.enter_context(tc.tile_pool(name="es", bufs=6))
        dn_pool = ctx.enter_context(tc.tile_pool(name="dn", bufs=4))
        rc_pool = ctx.enter_context(tc.tile_pool(name="rc", bufs=2))
        og_pool = ctx.enter_context(tc.tile_pool(name="og", bufs=4))
        pp = ctx.enter_context(tc.tile_pool(name="pp", bufs=1, space="PSUM"))

        # ---- resident tiles ----
        x4 = res.tile([P, 8, 2, SQ], f8e4, tag="x4")
        x5 = res.tile([P, 8, 2, SQ], f8e5, tag="x5")
        ck = res.tile([P, HG, SKV], f16, tag="ck")
        cv = res.tile([P, HG, 8, D_HEAD], f16, tag="cv")
        qkT = res.tile([P, 16, SQ], f16, tag="qkT")
        v_new = res.tile([P, 8, 1024], f16, tag="v_new")
        A4 = res.tile([P, HG, SQ], f8e4, tag="A4")
        A5 = res.tile([P, HG, SQ], f8e5, tag="A5")
        cosf = const.tile([P, SQ], f16, tag="cosf")
        sinn = const.tile([P, SQ], f16, tag="sinn")
        ones = const.tile([P, P], f16, tag="ones")
        nc.vector.memset(ones[:], 1.0)

        # PSUM tags: B0/B1 are [128,1024] (2 banks each), S0/S1/A0/A1 are
        # [128,512] (1 bank each). 8 banks total.
        def pbig(tag, name):
            return pp.tile([P, SQ], f32, tag=tag, name=name)

        def psml(tag, name):
            return pp.tile([P, 512], f32, tag=tag, name=name)

        # ---------------- phase V: V projection, fp8 DoubleRow ----------
        # out v_new[q, vcols]; stationary x-chunk [128,2,128q], moving wv
        # [128,2,512vc]. Two half-phases (c = vcol half) of 8 q-chunk groups.
        wv_tiles = {}

        def wv_load(c, j):
            w4t = wv_pool.tile([P, 2, 512], f8e4, tag="wv4")
            w5t = wv_pool.tile([P, 2, 512], f8e5, tag="wv5")
            nc.sync.dma_start(w4t[:], d_wv4[:, j, :, ts(c, 512)])
            nc.sync.dma_start(w5t[:], d_wv5[:, j, :, ts(c, 512)])
            wv_tiles[(c, j)] = (w4t, w5t)

        # first bite: small slice covering the first stationary chunks so the
        # PE's first matmul starts as early as possible
        nc.sync.dma_start(x4[:, 0:1, :, 0:256], d_x4[:, 0:1, :, 0:256])
        wv_load(0, 0)
        nc.sync.dma_start(x4[:, 0:1, :, 256:1024], d_x4[:, 0:1, :, 256:1024])
        nc.sync.dma_start(x5[:, 0:1], d_x5[:, 0:1])
        nc.sync.dma_start(x4[:, 1:2], d_x4[:, 1:2])
        nc.sync.dma_start(x5[:, 1:2], d_x5[:, 1:2])
        wv_load(0, 1)
        for c in range(2):
            big = [pbig("B0", f"ps_vb0_{c}"), pbig("B1", f"ps_vb1_{c}")]
            sml = [psml(t, f"ps_v{t}_{c}") for t in ("S0", "S1", "A0", "A1")]
            accs = [big[0][:, 0:512], big[0][:, 512:1024],
                    big[1][:, 0:512], big[1][:, 512:1024],
                    sml[0][:], sml[1][:], sml[2][:], sml[3][:]]
            for j in range(8):
                if c == 0 and j < 6:
                    nc.sync.dma_start(x4[:, j + 2:j + 3], d_x4[:, j + 2:j + 3])
                    nc.sync.dma_start(x5[:, j + 2:j + 3], d_x5[:, j + 2:j + 3])
                if j < 6:
                    wv_load(c, j + 2)
                elif c == 0:
                    wv_load(1, j - 6)
                if c == 1:
                    # first halves of the cache loads ride here; the rest
                    # goes in the qk(0) prologue window where DMA is idle
                    if j == 2:
                        nc.sync.dma_start(ck[:, 0:4], d_ck[:, 0:4])
                    elif j == 5:
                        nc.sync.dma_start(cv[:, 0:4], d_cv[:, 0:4])
                wv4t, wv5t = wv_tiles.pop((c, j))
                # x4*wv4 terms first so the PE can start before x5/wv5 land
                for st in range(8):
                    nc.tensor.matmul(accs[st], x4[:, j, :, ts(st, P)], wv4t[:],
                                     start=(j == 0), stop=False,
                                     perf_mode=DR)
                for st in range(8):
                    nc.tensor.matmul(accs[st], x4[:, j, :, ts(st, P)], wv5t[:],
                                     start=False, stop=False, perf_mode=DR)
                    nc.tensor.matmul(accs[st], x5[:, j, :, ts(st, P)], wv4t[:],
                                     start=False, stop=(j == 7),
                                     perf_mode=DR)
            for st in range(8):
                if st % 2 == 0:
                    nc.scalar.activation(v_new[:, st, ts(c, 512)], accs[st], COPY)
                else:
                    nc.vector.tensor_copy(v_new[:, st, ts(c, 512)], accs[st])


        # ---------------- QK projection helpers -------------------------
        # m in 0..15: output col-chunk (m<8: q of head m; m>=8: k of head m-8)
        # Each m-block: psum [128,1024] (B0 for q / B1 for k), 48 DoubleRow
        # matmuls (8 j-steps x 2 c-halves x 3 terms).
        def qk_load(m):
            w4t = wqk_pool.tile([P, 8, 2, P], f8e4, tag="wqk4")
            w5t = wqk_pool.tile([P, 8, 2, P], f8e5, tag="wqk5")
            nc.sync.dma_start(w4t[:], d_wqk4[:, m])
            nc.sync.dma_start(w5t[:], d_wqk5[:, m])
            return w4t, w5t

        def qk_mm(psm, w4t, w5t, j, c):
            xm4 = x4[:, j, :, ts(c, 512)]
            xm5 = x5[:, j, :, ts(c, 512)]
            out = psm[:, ts(c, 512)]
            for t, (wt, xt) in enumerate([(w4t[:, j], xm4), (w4t[:, j], xm5),
                                          (w5t[:, j], xm4)]):
                nc.tensor.matmul(out, wt, xt, start=(j == 0 and t == 0),
                                 stop=(j == 7 and t == 2), perf_mode=DR)

        def qk_rope(psm, m):
            # out[0:64] = p[0:64]*cos - p[64:]*sin ; out[64:] = p[64:]*cos + p[0:64]*sin
            t1 = tmp_pool.tile([P, SQ], f16, tag="t1")
            t0 = tmp_pool.tile([P, SQ], f16, tag="t0")
            nc.vector.tensor_tensor(t1[0:64, :], psm[64:128, :], sinn[0:64, :], MUL)
            nc.vector.tensor_tensor(t1[64:128, :], psm[0:64, :], sinn[64:128, :], MUL)
            nc.vector.tensor_tensor(t0[:], psm[:], cosf[:], MUL)
            nc.vector.tensor_tensor(qkT[:, m, :], t0[:], t1[:], ADD)

        # ---------------- attention slot helpers -------------------------
        es_q = {}
        dn_acc = {}

        def _kt(h, tt):
            return (ck[:, h, ts(tt, P)] if tt < 8
                    else qkT[:, 8 + h, ts(tt - 8, P)])

        def sc_exp_a(h, tt):
            ps0 = psml("S0", f"ps_sc0_{h}_{tt}")
            es = es_pool.tile([P, SQ], f16, tag="es", name=f"es{h}_{tt}")
            nc.tensor.matmul(ps0[:], _kt(h, tt), qkT[:, h, 0:512],
                             start=True, stop=True)
            nc.scalar.activation(es[:, 0:512], ps0[:], EXP, scale=ESCALE)
            return es

        def sc_exp_b(h, tt, es):
            ps1 = psml("S1", f"ps_sc1_{h}_{tt}")
            nc.tensor.matmul(ps1[:], _kt(h, tt), qkT[:, h, 512:1024],
                             start=True, stop=True)
            nc.scalar.activation(es[:, 512:1024], ps1[:], EXP, scale=ESCALE)
            es_q[(h, tt)] = es
            # dn accumulation on DVE (f16, SBUF-only -> fast mode)
            if tt == 0:
                dn_acc[h] = es
            else:
                nd = dn_pool.tile([P, SQ], f16, tag="dn", name=f"dn{h}_{tt}")
                nc.vector.tensor_tensor(nd[:], dn_acc[h][:], es[:], ADD)
                dn_acc[h] = nd

        def vtile(h, tt):
            if tt < 8:
                return cv[:, h, tt, :]
            return v_new[:, tt - 8, ts(h, P)]

        ps_av = [None, None]

        def av(h, tt):
            if tt == 0:
                ps_av[0] = psml("A0", f"ps_av0_{h}")
                ps_av[1] = psml("A1", f"ps_av1_{h}")
            es = es_q.pop((h, tt))
            for c in range(2):
                nc.tensor.matmul(ps_av[c][:], vtile(h, tt), es[:, ts(c, 512)],
                                 start=(tt == 0), stop=(tt == 15))

        def head_finish(h):
            # denominator broadcast matmul into B1 (free between K-blocks;
            # for the last head B1 is reserved for the O pre-fill, so use the
            # S banks which the sc pipeline no longer needs), then recip +
            # normalize + fp8 hi/lo split of attn.
            dn = dn_acc.pop(h)
            recip = rc_pool.tile([P, SQ], f16, tag="recip", name=f"recip{h}")
            t = tmp_pool.tile([P, SQ], f16, tag="attn", name=f"attn{h}")
            if h == 7:
                # epilogue: pipeline by column halves — the O phase's first
                # groups (q cols 0..511) only need the c=0 half of A4/A5, so
                # racing it through cuts the serial tail. ACT/DVE are idle.
                for c in range(2):
                    half = psml(("S0", "S1")[c], f"ps_dn7{c}")[:]
                    nc.tensor.matmul(half, ones[:], dn[:, ts(c, 512)],
                                     start=True, stop=True)
                    with nc.allow_low_precision(reason="f16 recip is ample"):
                        nc.vector.reciprocal(recip[:, ts(c, 512)], half)
                    nc.vector.tensor_tensor(t[:, ts(c, 512)], ps_av[c][:],
                                            recip[:, ts(c, 512)], MUL)
                    nc.scalar.activation(A4[:, h, ts(c, 512)],
                                         t[:, ts(c, 512)], COPY)
                    nc.vector.tensor_tensor(A5[:, h, ts(c, 512)],
                                            t[:, ts(c, 512)],
                                            A4[:, h, ts(c, 512)], SUB)
                return
            ps_dn = pbig("B1", f"ps_dn{h}")
            nc.tensor.matmul(ps_dn[:, 0:512], ones[:], dn[:, 0:512],
                             start=True, stop=True)
            nc.tensor.matmul(ps_dn[:, 512:1024], ones[:], dn[:, 512:1024],
                             start=True, stop=True)
            with nc.allow_low_precision(reason="softmax recip in f16 is ample"):
                nc.vector.reciprocal(recip[:, 0:512], ps_dn[:, 0:512])
                nc.vector.reciprocal(recip[:, 512:1024], ps_dn[:, 512:1024])
            for c in range(2):
                nc.vector.tensor_tensor(t[:, ts(c, 512)], ps_av[c][:],
                                        recip[:, ts(c, 512)], MUL)
            # steady state: ACT (exp) and DVE (dn/rope) are the tight
            # engines; the idle Pool engine absorbs the fp8 split
            nc.gpsimd.tensor_copy(A4[:, h, :], t[:])
            nc.gpsimd.tensor_tensor(A5[:, h, :], t[:], A4[:, h, :], SUB)

        # ---------------- prologue: qk(0) ---------------------------------
        w4t, w5t = qk_load(0)
        nc.sync.dma_start(cosf[:], d_cos[:])
        nc.sync.dma_start(sinn[:], d_sin[:])
        psq = pbig("B0", "ps_qk0")
        for j in range(8):
            for c in range(2):
                qk_mm(psq, w4t, w5t, j, c)
            if j == 1:
                nc.sync.dma_start(ck[:, 4:8], d_ck[:, 4:8])
            elif j == 4:
                nc.sync.dma_start(cv[:, 4:8], d_cv[:, 4:8])
        qk_rope(psq, 0)
        w4t, w5t = qk_load(8)
        psk = pbig("B1", "ps_qk8")
        for j in range(8):
            for c in range(2):
                qk_mm(psk, w4t, w5t, j, c)
        qk_rope(psk, 8)

        # ---------------- merged slot loop --------------------------------
        # slot s = 16h + tt. Per slot: sc+exp+dn for (h, tt); 6 qk matmuls of
        # head h+1 (m = h+1 during tt<8, m = 9+h during tt>=8); av lagged 3.
        qk_state = {}

        def qk_piece(h1, tt):
            # head h1's q block over tt=0..7 (6 mm per slot), k block 8..15
            m = h1 if tt < 8 else 8 + h1
            jj = tt % 8  # j index within the block
            if jj == 0:
                qk_state["w"] = qk_load(m)
                qk_state["ps"] = pbig("B0" if tt < 8 else "B1", f"ps_qk{m}")
            w4t, w5t = qk_state["w"]
            for c in range(2):
                qk_mm(qk_state["ps"], w4t, w5t, jj, c)
            if jj == 7:
                qk_rope(qk_state["ps"], m)

        wo_tiles = {}

        def wo_load(c4):
            wo4t = wo_pool.tile([P, HG, 512], f8e4, tag="wo4")
            wo5t = wo_pool.tile([P, HG, 512], f8e5, tag="wo5")
            nc.sync.dma_start(wo4t[:], d_wo4[:, :, ts(c4, 512)])
            nc.sync.dma_start(wo5t[:], d_wo5[:, :, ts(c4, 512)])
            wo_tiles[c4] = (wo4t, wo5t)

        # O-projection pre-fill: head 7's slots carry no qk matmuls, so the
        # PE idles behind ACT's exp stream there. Fill the holes with the
        # heads-0..5 partial terms (hp 0..2) of O c4=0's first two groups.
        o_pre = {}

        def o_pre_mm(k):
            # k in 0..35: (st4, hp, term); st4 0,1 on B0, st4 2,3 on B1
            st4, rem = divmod(k, 9)
            hp, t = divmod(rem, 3)
            if k == 0:
                o_pre["b0"] = pbig("B0", "ps_ob0_pre")
            elif k == 18:
                o_pre["b1"] = pbig("B1", "ps_ob1_pre")
            wo4t, wo5t = wo_tiles[0]
            big = o_pre["b0"] if st4 < 2 else o_pre["b1"]
            acc = big[:, ts(st4 % 2, 512)]
            a4s = A4[:, 2 * hp:2 * hp + 2, ts(st4, P)]
            a5s = A5[:, 2 * hp:2 * hp + 2, ts(st4, P)]
            w4s = wo4t[:, 2 * hp:2 * hp + 2, :]
            w5s = wo5t[:, 2 * hp:2 * hp + 2, :]
            a, w = [(a4s, w4s), (a4s, w5s), (a5s, w4s)][t]
            nc.tensor.matmul(acc, a, w, start=(rem == 0), stop=False,
                             perf_mode=DR)

        for s in range(128):
            h, tt = s // 16, s % 16
            # B0 groups (st4 0,1) fill slots 113-118; B1 groups (st4 2,3)
            # fill slots 119-127 where the PE otherwise idles behind exp
            if 113 <= s <= 118:
                for k in range(3 * (s - 113), 3 * (s - 112)):
                    o_pre_mm(k)
            elif 119 <= s <= 127:
                for k in range(18 + 2 * (s - 119), min(18 + 2 * (s - 118), 36)):
                    o_pre_mm(k)
            # qk before sc; the S1 half of sc goes after av so exp(s-1)'s
            # second half has drained its psum bank by then
            if h + 1 < 8:
                qk_piece(h + 1, tt)
            es = sc_exp_a(h, tt)
            if tt == 3 and h > 0:
                head_finish(h - 1)
            lag = s - 3
            if lag >= 0:
                av(lag // 16, lag % 16)
            sc_exp_b(h, tt, es)
            if s == 104:
                wo_load(0)
            elif s == 118:
                wo_load(1)
        for s in range(125, 128):
            av(s // 16, s % 16)
        head_finish(7)

        # ---------------- phase O: output projection, fp8 DoubleRow -------
        # out[q, col] = sum_h attn_h.T @ wo_h ; DoubleRow pairs heads
        # (contraction 256 = 2 heads x 128 dh per instruction).
        for c4 in range(4):
            wo4t, wo5t = wo_tiles.pop(c4)
            if c4 + 2 < 4:
                wo_load(c4 + 2)
            for grp in range(2):
                if grp == 0:
                    if c4 == 0:
                        big = [o_pre["b0"], o_pre["b1"]]
                    else:
                        big = [pbig("B0", f"ps_ob0_{c4}"),
                               pbig("B1", f"ps_ob1_{c4}")]
                    accs = [big[0][:, 0:512], big[0][:, 512:1024],
                            big[1][:, 0:512], big[1][:, 512:1024]]
                else:
                    accs = [psml(t, f"ps_o{t}_{c4}")[:]
                            for t in ("S0", "S1", "A0", "A1")]
                for st4 in range(4):
                    st = grp * 4 + st4
                    hp0 = 3 if (c4 == 0 and grp == 0) else 0
                    for hp in range(hp0, 4):
                        a4s = A4[:, 2 * hp:2 * hp + 2, ts(st, P)]
                        a5s = A5[:, 2 * hp:2 * hp + 2, ts(st, P)]
                        w4s = wo4t[:, 2 * hp:2 * hp + 2, :]
                        w5s = wo5t[:, 2 * hp:2 * hp + 2, :]
                        for t, (a, w) in enumerate([(a4s, w4s), (a4s, w5s),
                                                    (a5s, w4s)]):
                            nc.tensor.matmul(accs[st4], a, w,
                                             start=(hp == 0 and t == 0),
                                             stop=(hp == 3 and t == 2),
                                             perf_mode=DR)
                last = (c4 == 3 and grp == 1)
                for st4 in range(4):
                    st = grp * 4 + st4
                    og = og_pool.tile([P, 512], f16, tag="og",
                                      name=f"og{st}_{c4}")
                    if last and st4 == 3:
                        # final chunk: evict+store in column halves on both
                        # engines to shorten the end-of-kernel drain chain
                        nc.scalar.activation(og[:, 0:256], accs[st4][:, 0:256],
                                             COPY, scale=OUT_SCALE)
                        nc.sync.dma_start(
                            d_out[ts(st, P), c4 * 512:c4 * 512 + 256],
                            og[:, 0:256])
                        nc.vector.tensor_scalar_mul(og[:, 256:512],
                                                    accs[st4][:, 256:512],
                                                    OUT_SCALE)
                        nc.sync.dma_start(
                            d_out[ts(st, P), c4 * 512 + 256:c4 * 512 + 512],
                            og[:, 256:512])
                        continue
                    if st4 % 2 == 0:
                        nc.scalar.activation(og[:], accs[st4], COPY,
                                             scale=OUT_SCALE)
                    else:
                        nc.vector.tensor_scalar_mul(og[:], accs[st4], OUT_SCALE)
                    nc.sync.dma_start(d_out[ts(st, P), ts(c4, 512)], og[:])

    nc.compile()
    return nc


def _get_module():
    if "nc" not in _BUILD_CACHE:
        _BUILD_CACHE["nc"] = build_module()
    return _BUILD_CACHE["nc"]


def _split8(a):
    hi = a.astype(E4)
    lo = (a - hi.astype(np.float32)).astype(E5)
    return hi, lo


def _pack_rows(a):
    """[2048 rows, cols] -> [128, 8, 2, cols]: row k = 256*j + 128*i + p."""
    cols = a.shape[1]
    return np.ascontiguousarray(
        a.reshape(8, 2, P, cols).transpose(2, 0, 1, 3))


def _prep_core_inputs(x, cache_k, cache_v, w_qkv, w_o, cosf, sinn, b, g):
    heads = list(range(g * HG, (g + 1) * HG))
    qcols = np.concatenate([np.arange(384 * H, 384 * H + 128) for H in heads])
    kcols = qcols + 128
    vcols = qcols + 256

    xt = x[b].T * SX                       # [2048 k, 1024 q] scaled
    xh, xl = _split8(xt.astype(np.float32))
    x4 = _pack_rows(xh.astype(np.float32)).astype(E4)
    x5 = _pack_rows(xl.astype(np.float32)).astype(E5)

    w_qk = w_qkv[:, np.concatenate([qcols, kcols])] * SW    # [2048, 2048]
    wh, wl = _split8(w_qk.astype(np.float32))
    # [128, 16 m, 8 j, 2 i, 128 c]
    wqk4 = np.ascontiguousarray(
        _pack_rows(wh.astype(np.float32)).reshape(P, 8, 2, 16, P)
        .transpose(0, 3, 1, 2, 4)).astype(E4)
    wqk5 = np.ascontiguousarray(
        _pack_rows(wl.astype(np.float32)).reshape(P, 8, 2, 16, P)
        .transpose(0, 3, 1, 2, 4)).astype(E5)

    w_v = w_qkv[:, vcols] * SW                              # [2048, 1024]
    wvh, wvl = _split8(w_v.astype(np.float32))
    wv4 = _pack_rows(wvh.astype(np.float32)).astype(E4)
    wv5 = _pack_rows(wvl.astype(np.float32)).astype(E5)

    ckt = np.ascontiguousarray(
        cache_k[b, heads].transpose(2, 0, 1) * SCL).astype(F16)
    cvt = np.ascontiguousarray(
        cache_v[b, heads].reshape(HG, 8, P, D_HEAD).transpose(2, 0, 1, 3)
        * SCL).astype(F16)

    rows = np.concatenate([np.arange(P * H, P * (H + 1)) for H in heads])
    wot = w_o[rows] * SW                                    # [1024, 2048]
    woh, wol = _split8(wot.astype(np.float32))
    # [128 dh, 8 head, 2048]
    wo4 = np.ascontiguousarray(
        woh.astype(np.float32).reshape(HG, P, 2048).transpose(1, 0, 2)).astype(E4)
    wo5 = np.ascontiguousarray(
        wol.astype(np.float32).reshape(HG, P, 2048).transpose(1, 0, 2)).astype(E5)

    return {"x4": x4, "x5": x5, "wqk4": wqk4, "wqk5": wqk5,
            "wv4": wv4, "wv5": wv5, "ck": ckt, "cv": cvt,
            "wo4": wo4, "wo5": wo5, "cosf": cosf, "sinn": sinn}


def kernel(x, cache_k, cache_v, w_qkv, w_o, trace=False):
    from concourse import bass_utils

    nc = _get_module()
    cosf, sinn = _rope_tables()
    x = np.asarray(x); cache_k = np.asarray(cache_k); cache_v = np.asarray(cache_v)
    w_qkv = np.asarray(w_qkv); w_o = np.asarray(w_o)

    in_maps = []
    for core in range(N_CORES):
        b, g = core // 2, core % 2
        in_maps.append(_prep_core_inputs(x, cache_k, cache_v, w_qkv, w_o,
                                         cosf, sinn, b, g))

    res = bass_utils.run_bass_kernel_spmd(nc, in_maps,
                                          core_ids=list(range(N_CORES)),
                                          trace=trace)
    _BUILD_CACHE["last_result"] = res
    out = np.zeros((B, SQ, D_MODEL), dtype=np.float32)
    for core in range(N_CORES):
        out[core // 2] += res.results[core]["out"].astype(np.float32)
    return out


if __name__ == "__main__":
    rng = np.random.default_rng(0)
    ins = {
        "x": rng.standard_normal((B, SQ, D_MODEL), dtype=np.float32),
        "cache_k": rng.standard_normal((B, N_HEADS, SKV, D_HEAD), dtype=np.float32),
        "cache_v": rng.standard_normal((B, N_HEADS, SKV, D_HEAD), dtype=np.float32),
        "w_qkv": rng.standard_normal((D_MODEL, 3 * D_MODEL), dtype=np.float32) * D_MODEL ** -0.5,
        "w_o": rng.standard_normal((D_MODEL, D_MODEL), dtype=np.float32) * D_MODEL ** -0.5,
    }
    out = kernel(**ins)
    print("out", out.shape, out.dtype, float(np.abs(out).max()))


# revision 5
# speedup vs baseline: 1.0167x; 1.0004x over previous
"""Tensor-parallel attention kernel for TRN2 (8 NeuronCores), v2.

Sharding: 2D grid — 4 batches x 2 head-groups (8 heads each). Core c handles
batch b = c // 2, head-group g = c % 2.

Key speedups over v1:
  1. All projections (QKV and O) run as fp8 DoubleRow matmuls with an
     e4m3-hi + e5m2-lo 3-term decomposition (X*W ~ Xh*Wh + Xh*Wl + Xl*Wh).
     DoubleRow contracts 256 rows per instruction at 0.5 cycles/row, so the
     3-term group costs 0.75x one f16 matmul while keeping ~0.3% error.
  2. Softmax denominator via DVE elementwise accumulation of exp tiles plus
     one small ones-matmul per head (replaces the per-slot ones-matmul
     ridealong that cost a full extra AV pass on the PE).
  3. QK projection matmuls are interleaved into the attention slot loop so
     the scalar engine (exp) is never the exposed critical path.
  4. f16 output stores; wo streamed ahead per c4 block.

Scales: x is pre-scaled by SX=4, all weights by SW=64 (keeps fp8 hi parts in
the e4m3 normal range). cache_k/cache_v are pre-scaled by SX*SW so scores
carry a uniform (SX*SW)^2 factor folded into the exp scale; the O-projection
eviction divides by SX*SW*SW.
"""

from contextlib import ExitStack

import numpy as np
import ml_dtypes

B, SQ, SKV = 4, 1024, 1024
D_MODEL = 2048
N_HEADS = 16
D_HEAD = 128
ROPE_THETA = 10000.0
N_CORES = 8
HG = 8  # heads per core
P = 128

F16 = np.float16
E4 = ml_dtypes.float8_e4m3
E5 = ml_dtypes.float8_e5m2

SX = 4.0
SW = 64.0
SCL = SX * SW          # scale carried by q, k, v, attn
OUT_SCALE = 1.0 / (SCL * SW)   # applied at O eviction

_BUILD_CACHE = {}


def _rope_tables():
    """cosf/sinn [128, 1024] f16 for new-token positions SKV + s."""
    inv_freq = 1.0 / (ROPE_THETA ** (np.arange(0, D_HEAD, 2, dtype=np.float32) / D_HEAD))
    pos = (SKV + np.arange(SQ, dtype=np.float32))
    ang = pos[:, None] * inv_freq[None, :]           # [S, 64]
    cos = np.cos(ang).astype(np.float32).T           # [64, S]
    sin = np.sin(ang).astype(np.float32).T
    cosf = np.concatenate([cos, cos], axis=0)        # [128, S]
    sinn = np.concatenate([-sin, sin], axis=0)       # [128, S]
    return (np.ascontiguousarray(cosf).astype(F16),
            np.ascontiguousarray(sinn).astype(F16))


def build_module():
    import concourse.mybir as mybir
    import concourse.tile as tile
    from concourse import bacc
    from concourse.bass import ts

    f32 = mybir.dt.float32
    f16 = mybir.dt.float16
    f8e4 = mybir.dt.float8e4
    f8e5 = mybir.dt.float8e5
    DR = mybir.MatmulPerfMode.DoubleRow

    nc = bacc.Bacc("TRN2", target_bir_lowering=False, debug=False,
                   num_devices=N_CORES)

    # fp8 packed tensors use sub-row packing [P, j, 2, cols]:
    # contraction row k = 256*j + 128*i + p.
    d_x4 = nc.dram_tensor("x4", [P, 8, 2, SQ], f8e4, kind="ExternalInput").ap()
    d_x5 = nc.dram_tensor("x5", [P, 8, 2, SQ], f8e5, kind="ExternalInput").ap()
    d_wqk4 = nc.dram_tensor("wqk4", [P, 16, 8, 2, P], f8e4, kind="ExternalInput").ap()
    d_wqk5 = nc.dram_tensor("wqk5", [P, 16, 8, 2, P], f8e5, kind="ExternalInput").ap()
    d_wv4 = nc.dram_tensor("wv4", [P, 8, 2, 1024], f8e4, kind="ExternalInput").ap()
    d_wv5 = nc.dram_tensor("wv5", [P, 8, 2, 1024], f8e5, kind="ExternalInput").ap()
    d_ck = nc.dram_tensor("ck", [P, HG, SKV], f16, kind="ExternalInput").ap()
    d_cv = nc.dram_tensor("cv", [P, HG, 8, D_HEAD], f16, kind="ExternalInput").ap()
    # wo packed for DoubleRow over head pairs: [P(dh), 8 heads, 2048 cols]
    d_wo4 = nc.dram_tensor("wo4", [P, HG, 2048], f8e4, kind="ExternalInput").ap()
    d_wo5 = nc.dram_tensor("wo5", [P, HG, 2048], f8e5, kind="ExternalInput").ap()
    d_cos = nc.dram_tensor("cosf", [P, SQ], f16, kind="ExternalInput").ap()
    d_sin = nc.dram_tensor("sinn", [P, SQ], f16, kind="ExternalInput").ap()
    d_out = nc.dram_tensor("out", [SQ, D_MODEL], f16, kind="ExternalOutput").ap()

    EXP = mybir.ActivationFunctionType.Exp
    COPY = mybir.ActivationFunctionType.Copy
    MUL = mybir.AluOpType.mult
    ADD = mybir.AluOpType.add
    SUB = mybir.AluOpType.subtract
    ESCALE = float(D_HEAD) ** -0.5 / (SCL * SCL)

    with tile.TileContext(nc) as tc, ExitStack() as ctx:
        const = ctx.enter_context(tc.tile_pool(name="const", bufs=1))
        res = ctx.enter_context(tc.tile_pool(name="res", bufs=1))
        wqk_pool = ctx.enter_context(tc.tile_pool(name="wqk", bufs=3))
        wv_pool = ctx.enter_context(tc.tile_pool(name="wv", bufs=3))
        wo_pool = ctx.enter_context(tc.tile_pool(name="wo", bufs=2))
        tmp_pool = ctx.enter_context(tc.tile_pool(name="tmp", bufs=2))
        es_pool = ctx.enter_context(tc.tile_pool(name="es", bufs=6))
        dn_pool = ctx.enter_context(tc.tile_pool(name="dn", bufs=4))
        rc_pool = ctx.enter_context(tc.tile_pool(name="rc", bufs=2))
        og_pool = ctx.enter_context(tc.tile_pool(name="og", bufs=4))
        pp = ctx.enter_context(tc.tile_pool(name="pp", bufs=1, space="PSUM"))

        # ---- resident tiles ----
        x4 = res.tile([P, 8, 2, SQ], f8e4, tag="x4")
        x5 = res.tile([P, 8, 2, SQ], f8e5, tag="x5")
        ck = res.tile([P, HG, SKV], f16, tag="ck")
        cv = res.tile([P, HG, 8, D_HEAD], f16, tag="cv")
        qkT = res.tile([P, 16, SQ], f16, tag="qkT")
        v_new = res.tile([P, 8, 1024], f16, tag="v_new")
        A4 = res.tile([P, HG, SQ], f8e4, tag="A4")
        A5 = res.tile([P, HG, SQ], f8e5, tag="A5")
        cosf = const.tile([P, SQ], f16, tag="cosf")
        sinn = const.tile([P, SQ], f16, tag="sinn")
        ones = const.tile([P, P], f16, tag="ones")
        nc.vector.memset(ones[:], 1.0)

        # PSUM tags: B0/B1 are [128,1024] (2 banks each), S0/S1/A0/A1 are
        # [128,512] (1 bank each). 8 banks total.
        def pbig(tag, name):
            return pp.tile([P, SQ], f32, tag=tag, name=name)

        def psml(tag, name):
            return pp.tile([P, 512], f32, tag=tag, name=name)

        # ---------------- phase V: V projection, fp8 DoubleRow ----------
        # out v_new[q, vcols]; stationary x-chunk [128,2,128q], moving wv
        # [128,2,512vc]. Two half-phases (c = vcol half) of 8 q-chunk groups.
        wv_tiles = {}

        def wv_load(c, j):
            w4t = wv_pool.tile([P, 2, 512], f8e4, tag="wv4")
            w5t = wv_pool.tile([P, 2, 512], f8e5, tag="wv5")
            nc.sync.dma_start(w4t[:], d_wv4[:, j, :, ts(c, 512)])
            nc.sync.dma_start(w5t[:], d_wv5[:, j, :, ts(c, 512)])
            wv_tiles[(c, j)] = (w4t, w5t)

        # first bite: small slice covering the first stationary chunks so the
        # PE's first matmul starts as early as possible
        nc.sync.dma_start(x4[:, 0:1, :, 0:256], d_x4[:, 0:1, :, 0:256])
        wv_load(0, 0)
        nc.sync.dma_start(x4[:, 0:1, :, 256:1024], d_x4[:, 0:1, :, 256:1024])
        nc.sync.dma_start(x5[:, 0:1], d_x5[:, 0:1])
        nc.sync.dma_start(x4[:, 1:2], d_x4[:, 1:2])
        nc.sync.dma_start(x5[:, 1:2], d_x5[:, 1:2])
        wv_load(0, 1)
        for c in range(2):
            big = [pbig("B0", f"ps_vb0_{c}"), pbig("B1", f"ps_vb1_{c}")]
            sml = [psml(t, f"ps_v{t}_{c}") for t in ("S0", "S1", "A0", "A1")]
            accs = [big[0][:, 0:512], big[0][:, 512:1024],
                    big[1][:, 0:512], big[1][:, 512:1024],
                    sml[0][:], sml[1][:], sml[2][:], sml[3][:]]
            for j in range(8):
                if c == 0 and j < 6:
                    nc.sync.dma_start(x4[:, j + 2:j + 3], d_x4[:, j + 2:j + 3])
                    nc.sync.dma_start(x5[:, j + 2:j + 3], d_x5[:, j + 2:j + 3])
                if j < 6:
                    wv_load(c, j + 2)
                elif c == 0:
                    wv_load(1, j - 6)

                wv4t, wv5t = wv_tiles.pop((c, j))
                # x4*wv4 terms first so the PE can start before x5/wv5 land
                for st in range(8):
                    nc.tensor.matmul(accs[st], x4[:, j, :, ts(st, P)], wv4t[:],
                                     start=(j == 0), stop=False,
                                     perf_mode=DR)
                for st in range(8):
                    nc.tensor.matmul(accs[st], x4[:, j, :, ts(st, P)], wv5t[:],
                                     start=False, stop=False, perf_mode=DR)
                    nc.tensor.matmul(accs[st], x5[:, j, :, ts(st, P)], wv4t[:],
                                     start=False, stop=(j == 7),
                                     perf_mode=DR)
            for st in range(8):
                if st % 2 == 0:
                    nc.scalar.activation(v_new[:, st, ts(c, 512)], accs[st], COPY)
                else:
                    nc.vector.tensor_copy(v_new[:, st, ts(c, 512)], accs[st])


        # ---------------- QK projection helpers -------------------------
        # m in 0..15: output col-chunk (m<8: q of head m; m>=8: k of head m-8)
        # Each m-block: psum [128,1024] (B0 for q / B1 for k), 48 DoubleRow
        # matmuls (8 j-steps x 2 c-halves x 3 terms).
        def qk_load(m):
            w4t = wqk_pool.tile([P, 8, 2, P], f8e4, tag="wqk4")
            w5t = wqk_pool.tile([P, 8, 2, P], f8e5, tag="wqk5")
            nc.sync.dma_start(w4t[:], d_wqk4[:, m])
            nc.sync.dma_start(w5t[:], d_wqk5[:, m])
            return w4t, w5t

        def qk_mm(psm, w4t, w5t, j, c):
            xm4 = x4[:, j, :, ts(c, 512)]
            xm5 = x5[:, j, :, ts(c, 512)]
            out = psm[:, ts(c, 512)]
            for t, (wt, xt) in enumerate([(w4t[:, j], xm4), (w4t[:, j], xm5),
                                          (w5t[:, j], xm4)]):
                nc.tensor.matmul(out, wt, xt, start=(j == 0 and t == 0),
                                 stop=(j == 7 and t == 2), perf_mode=DR)

        def qk_rope(psm, m):
            # out[0:64] = p[0:64]*cos - p[64:]*sin ; out[64:] = p[64:]*cos + p[0:64]*sin
            t1 = tmp_pool.tile([P, SQ], f16, tag="t1")
            t0 = tmp_pool.tile([P, SQ], f16, tag="t0")
            nc.vector.tensor_tensor(t1[0:64, :], psm[64:128, :], sinn[0:64, :], MUL)
            nc.vector.tensor_tensor(t1[64:128, :], psm[0:64, :], sinn[64:128, :], MUL)
            nc.vector.tensor_tensor(t0[:], psm[:], cosf[:], MUL)
            nc.vector.tensor_tensor(qkT[:, m, :], t0[:], t1[:], ADD)

        # ---------------- attention slot helpers -------------------------
        es_q = {}
        dn_acc = {}

        def _kt(h, tt):
            return (ck[:, h, ts(tt, P)] if tt < 8
                    else qkT[:, 8 + h, ts(tt - 8, P)])

        def sc_exp_a(h, tt):
            ps0 = psml("S0", f"ps_sc0_{h}_{tt}")
            es = es_pool.tile([P, SQ], f16, tag="es", name=f"es{h}_{tt}")
            nc.tensor.matmul(ps0[:], _kt(h, tt), qkT[:, h, 0:512],
                             start=True, stop=True)
            nc.scalar.activation(es[:, 0:512], ps0[:], EXP, scale=ESCALE)
            return es

        def sc_exp_b(h, tt, es):
            ps1 = psml("S1", f"ps_sc1_{h}_{tt}")
            nc.tensor.matmul(ps1[:], _kt(h, tt), qkT[:, h, 512:1024],
                             start=True, stop=True)
            nc.scalar.activation(es[:, 512:1024], ps1[:], EXP, scale=ESCALE)
            es_q[(h, tt)] = es
            # dn accumulation on DVE (f16, SBUF-only -> fast mode)
            if tt == 0:
                dn_acc[h] = es
            else:
                nd = dn_pool.tile([P, SQ], f16, tag="dn", name=f"dn{h}_{tt}")
                nc.vector.tensor_tensor(nd[:], dn_acc[h][:], es[:], ADD)
                dn_acc[h] = nd

        def vtile(h, tt):
            if tt < 8:
                return cv[:, h, tt, :]
            return v_new[:, tt - 8, ts(h, P)]

        ps_av = [None, None]

        def av(h, tt):
            if tt == 0:
                ps_av[0] = psml("A0", f"ps_av0_{h}")
                ps_av[1] = psml("A1", f"ps_av1_{h}")
            es = es_q.pop((h, tt))
            for c in range(2):
                nc.tensor.matmul(ps_av[c][:], vtile(h, tt), es[:, ts(c, 512)],
                                 start=(tt == 0), stop=(tt == 15))

        def head_finish(h):
            # denominator broadcast matmul into B1 (free between K-blocks;
            # for the last head B1 is reserved for the O pre-fill, so use the
            # S banks which the sc pipeline no longer needs), then recip +
            # normalize + fp8 hi/lo split of attn.
            dn = dn_acc.pop(h)
            recip = rc_pool.tile([P, SQ], f16, tag="recip", name=f"recip{h}")
            t = tmp_pool.tile([P, SQ], f16, tag="attn", name=f"attn{h}")
            if h == 7:
                # epilogue: pipeline by column halves — the O phase's first
                # groups (q cols 0..511) only need the c=0 half of A4/A5, so
                # racing it through cuts the serial tail. ACT/DVE are idle.
                for c in range(2):
                    half = psml(("S0", "S1")[c], f"ps_dn7{c}")[:]
                    nc.tensor.matmul(half, ones[:], dn[:, ts(c, 512)],
                                     start=True, stop=True)
                    with nc.allow_low_precision(reason="f16 recip is ample"):
                        nc.vector.reciprocal(recip[:, ts(c, 512)], half)
                    nc.vector.tensor_tensor(t[:, ts(c, 512)], ps_av[c][:],
                                            recip[:, ts(c, 512)], MUL)
                    nc.scalar.activation(A4[:, h, ts(c, 512)],
                                         t[:, ts(c, 512)], COPY)
                    nc.vector.tensor_tensor(A5[:, h, ts(c, 512)],
                                            t[:, ts(c, 512)],
                                            A4[:, h, ts(c, 512)], SUB)
                return
            ps_dn = pbig("B1", f"ps_dn{h}")
            nc.tensor.matmul(ps_dn[:, 0:512], ones[:], dn[:, 0:512],
                             start=True, stop=True)
            nc.tensor.matmul(ps_dn[:, 512:1024], ones[:], dn[:, 512:1024],
                             start=True, stop=True)
            with nc.allow_low_precision(reason="softmax recip in f16 is ample"):
                nc.vector.reciprocal(recip[:, 0:512], ps_dn[:, 0:512])
                nc.vector.reciprocal(recip[:, 512:1024], ps_dn[:, 512:1024])
            for c in range(2):
                nc.vector.tensor_tensor(t[:, ts(c, 512)], ps_av[c][:],
                                        recip[:, ts(c, 512)], MUL)
            # steady state: ACT (exp) and DVE (dn/rope) are the tight
            # engines; the idle Pool engine absorbs the fp8 split
            nc.gpsimd.tensor_copy(A4[:, h, :], t[:])
            nc.gpsimd.tensor_tensor(A5[:, h, :], t[:], A4[:, h, :], SUB)

        # ---------------- prologue: qk(0) ---------------------------------
        # both weight tiles load first, then all cache/rope-table loads ride
        # this window's idle DMA behind them
        w4t, w5t = qk_load(0)
        w4t8, w5t8 = qk_load(8)
        nc.sync.dma_start(cosf[:], d_cos[:])
        nc.sync.dma_start(sinn[:], d_sin[:])
        psq = pbig("B0", "ps_qk0")
        for j in range(8):
            for c in range(2):
                qk_mm(psq, w4t, w5t, j, c)
            if j == 1:
                nc.sync.dma_start(ck[:, 0:4], d_ck[:, 0:4])
            elif j == 3:
                nc.sync.dma_start(ck[:, 4:8], d_ck[:, 4:8])
            elif j == 5:
                nc.sync.dma_start(cv[:, 0:4], d_cv[:, 0:4])
        qk_rope(psq, 0)
        psk = pbig("B1", "ps_qk8")
        for j in range(8):
            for c in range(2):
                qk_mm(psk, w4t8, w5t8, j, c)
            if j == 1:
                nc.sync.dma_start(cv[:, 4:8], d_cv[:, 4:8])
        qk_rope(psk, 8)

        # ---------------- merged slot loop --------------------------------
        # slot s = 16h + tt. Per slot: sc+exp+dn for (h, tt); 6 qk matmuls of
        # head h+1 (m = h+1 during tt<8, m = 9+h during tt>=8); av lagged 3.
        qk_state = {}

        def qk_piece(h1, tt):
            # head h1's q block over tt=0..7 (6 mm per slot), k block 8..15
            m = h1 if tt < 8 else 8 + h1
            jj = tt % 8  # j index within the block
            if jj == 0:
                qk_state["w"] = qk_load(m)
                qk_state["ps"] = pbig("B0" if tt < 8 else "B1", f"ps_qk{m}")
            w4t, w5t = qk_state["w"]
            for c in range(2):
                qk_mm(qk_state["ps"], w4t, w5t, jj, c)
            if jj == 7:
                qk_rope(qk_state["ps"], m)

        wo_tiles = {}

        def wo_load(c4):
            wo4t = wo_pool.tile([P, HG, 512], f8e4, tag="wo4")
            wo5t = wo_pool.tile([P, HG, 512], f8e5, tag="wo5")
            nc.sync.dma_start(wo4t[:], d_wo4[:, :, ts(c4, 512)])
            nc.sync.dma_start(wo5t[:], d_wo5[:, :, ts(c4, 512)])
            wo_tiles[c4] = (wo4t, wo5t)

        # O-projection pre-fill: head 7's slots carry no qk matmuls, so the
        # PE idles behind ACT's exp stream there. Fill the holes with the
        # heads-0..5 partial terms (hp 0..2) of O c4=0's first two groups.
        o_pre = {}

        def o_pre_mm(k):
            # k in 0..35: (st4, hp, term); st4 0,1 on B0, st4 2,3 on B1
            st4, rem = divmod(k, 9)
            hp, t = divmod(rem, 3)
            if k == 0:
                o_pre["b0"] = pbig("B0", "ps_ob0_pre")
            elif k == 18:
                o_pre["b1"] = pbig("B1", "ps_ob1_pre")
            wo4t, wo5t = wo_tiles[0]
            big = o_pre["b0"] if st4 < 2 else o_pre["b1"]
            acc = big[:, ts(st4 % 2, 512)]
            a4s = A4[:, 2 * hp:2 * hp + 2, ts(st4, P)]
            a5s = A5[:, 2 * hp:2 * hp + 2, ts(st4, P)]
            w4s = wo4t[:, 2 * hp:2 * hp + 2, :]
            w5s = wo5t[:, 2 * hp:2 * hp + 2, :]
            a, w = [(a4s, w4s), (a4s, w5s), (a5s, w4s)][t]
            nc.tensor.matmul(acc, a, w, start=(rem == 0), stop=False,
                             perf_mode=DR)

        for s in range(128):
            h, tt = s // 16, s % 16
            # B0 groups (st4 0,1) fill slots 113-118; B1 groups (st4 2,3)
            # fill slots 119-127 where the PE otherwise idles behind exp
            if 113 <= s <= 118:
                for k in range(3 * (s - 113), 3 * (s - 112)):
                    o_pre_mm(k)
            elif 119 <= s <= 127:
                for k in range(18 + 2 * (s - 119), min(18 + 2 * (s - 118), 36)):
                    o_pre_mm(k)
            # qk before sc; the S1 half of sc goes after av so exp(s-1)'s
            # second half has drained its psum bank by then
            if h + 1 < 8:
                qk_piece(h + 1, tt)
            es = sc_exp_a(h, tt)
            if tt == 3 and h > 0:
                head_finish(h - 1)
            lag = s - 3
            if lag >= 0:
                av(lag // 16, lag % 16)
            sc_exp_b(h, tt, es)
            if s == 104:
                wo_load(0)
            elif s == 118:
                wo_load(1)
        for s in range(125, 128):
            av(s // 16, s % 16)
        head_finish(7)

        # ---------------- phase O: output projection, fp8 DoubleRow -------
        # out[q, col] = sum_h attn_h.T @ wo_h ; DoubleRow pairs heads
        # (contraction 256 = 2 heads x 128 dh per instruction).
        for c4 in range(4):
            wo4t, wo5t = wo_tiles.pop(c4)
            if c4 + 2 < 4:
                wo_load(c4 + 2)
            for grp in range(2):
                if grp == 0:
                    if c4 == 0:
                        big = [o_pre["b0"], o_pre["b1"]]
                    else:
                        big = [pbig("B0", f"ps_ob0_{c4}"),
                               pbig("B1", f"ps_ob1_{c4}")]
                    accs = [big[0][:, 0:512], big[0][:, 512:1024],
                            big[1][:, 0:512], big[1][:, 512:1024]]
                else:
                    accs = [psml(t, f"ps_o{t}_{c4}")[:]
                            for t in ("S0", "S1", "A0", "A1")]
                for st4 in range(4):
                    st = grp * 4 + st4
                    hp0 = 3 if (c4 == 0 and grp == 0) else 0
                    for hp in range(hp0, 4):
                        a4s = A4[:, 2 * hp:2 * hp + 2, ts(st, P)]
                        a5s = A5[:, 2 * hp:2 * hp + 2, ts(st, P)]
                        w4s = wo4t[:, 2 * hp:2 * hp + 2, :]
                        w5s = wo5t[:, 2 * hp:2 * hp + 2, :]
                        for t, (a, w) in enumerate([(a4s, w4s), (a4s, w5s),
                                                    (a5s, w4s)]):
                            nc.tensor.matmul(accs[st4], a, w,
                                             start=(hp == 0 and t == 0),
                                             stop=(hp == 3 and t == 2),
                                             perf_mode=DR)
                last = (c4 == 3 and grp == 1)
                for st4 in range(4):
                    st = grp * 4 + st4
                    og = og_pool.tile([P, 512], f16, tag="og",
                                      name=f"og{st}_{c4}")
                    if last and st4 == 3:
                        # final chunk: evict+store in column halves on both
                        # engines to shorten the end-of-kernel drain chain
                        nc.scalar.activation(og[:, 0:256], accs[st4][:, 0:256],
                                             COPY, scale=OUT_SCALE)
                        nc.sync.dma_start(
                            d_out[ts(st, P), c4 * 512:c4 * 512 + 256],
                            og[:, 0:256])
                        nc.vector.tensor_scalar_mul(og[:, 256:512],
                                                    accs[st4][:, 256:512],
                                                    OUT_SCALE)
                        nc.sync.dma_start(
                            d_out[ts(st, P), c4 * 512 + 256:c4 * 512 + 512],
                            og[:, 256:512])
                        continue
                    if st4 % 2 == 0:
                        nc.scalar.activation(og[:], accs[st4], COPY,
                                             scale=OUT_SCALE)
                    else:
                        nc.vector.tensor_scalar_mul(og[:], accs[st4], OUT_SCALE)
                    nc.sync.dma_start(d_out[ts(st, P), ts(c4, 512)], og[:])

    nc.compile()
    return nc


def _get_module():
    if "nc" not in _BUILD_CACHE:
        _BUILD_CACHE["nc"] = build_module()
    return _BUILD_CACHE["nc"]


def _split8(a):
    hi = a.astype(E4)
    lo = (a - hi.astype(np.float32)).astype(E5)
    return hi, lo


def _pack_rows(a):
    """[2048 rows, cols] -> [128, 8, 2, cols]: row k = 256*j + 128*i + p."""
    cols = a.shape[1]
    return np.ascontiguousarray(
        a.reshape(8, 2, P, cols).transpose(2, 0, 1, 3))


def _prep_core_inputs(x, cache_k, cache_v, w_qkv, w_o, cosf, sinn, b, g):
    heads = list(range(g * HG, (g + 1) * HG))
    qcols = np.concatenate([np.arange(384 * H, 384 * H + 128) for H in heads])
    kcols = qcols + 128
    vcols = qcols + 256

    xt = x[b].T * SX                       # [2048 k, 1024 q] scaled
    xh, xl = _split8(xt.astype(np.float32))
    x4 = _pack_rows(xh.astype(np.float32)).astype(E4)
    x5 = _pack_rows(xl.astype(np.float32)).astype(E5)

    w_qk = w_qkv[:, np.concatenate([qcols, kcols])] * SW    # [2048, 2048]
    wh, wl = _split8(w_qk.astype(np.float32))
    # [128, 16 m, 8 j, 2 i, 128 c]
    wqk4 = np.ascontiguousarray(
        _pack_rows(wh.astype(np.float32)).reshape(P, 8, 2, 16, P)
        .transpose(0, 3, 1, 2, 4)).astype(E4)
    wqk5 = np.ascontiguousarray(
        _pack_rows(wl.astype(np.float32)).reshape(P, 8, 2, 16, P)
        .transpose(0, 3, 1, 2, 4)).astype(E5)

    w_v = w_qkv[:, vcols] * SW                              # [2048, 1024]
    wvh, wvl = _split8(w_v.astype(np.float32))
    wv4 = _pack_rows(wvh.astype(np.float32)).astype(E4)
    wv5 = _pack_rows(wvl.astype(np.float32)).astype(E5)

    ckt = np.ascontiguousarray(
        cache_k[b, heads].transpose(2, 0, 1) * SCL).astype(F16)
    cvt = np.ascontiguousarray(
        cache_v[b, heads].reshape(HG, 8, P, D_HEAD).transpose(2, 0, 1, 3)
        * SCL).astype(F16)

    rows = np.concatenate([np.arange(P * H, P * (H + 1)) for H in heads])
    wot = w_o[rows] * SW                                    # [1024, 2048]
    woh, wol = _split8(wot.astype(np.float32))
    # [128 dh, 8 head, 2048]
    wo4 = np.ascontiguousarray(
        woh.astype(np.float32).reshape(HG, P, 2048).transpose(1, 0, 2)).astype(E4)
    wo5 = np.ascontiguousarray(
        wol.astype(np.float32).reshape(HG, P, 2048).transpose(1, 0, 2)).astype(E5)

    return {"x4": x4, "x5": x5, "wqk4": wqk4, "wqk5": wqk5,
            "wv4": wv4, "wv5": wv5, "ck": ckt, "cv": cvt,
            "wo4": wo4, "wo5": wo5, "cosf": cosf, "sinn": sinn}


def kernel(x, cache_k, cache_v, w_qkv, w_o, trace=False):
    from concourse import bass_utils

    nc = _get_module()
    cosf, sinn = _rope_tables()
    x = np.asarray(x); cache_k = np.asarray(cache_k); cache_v = np.asarray(cache_v)
    w_qkv = np.asarray(w_qkv); w_o = np.asarray(w_o)

    in_maps = []
    for core in range(N_CORES):
        b, g = core // 2, core % 2
        in_maps.append(_prep_core_inputs(x, cache_k, cache_v, w_qkv, w_o,
                                         cosf, sinn, b, g))

    res = bass_utils.run_bass_kernel_spmd(nc, in_maps,
                                          core_ids=list(range(N_CORES)),
                                          trace=trace)
    _BUILD_CACHE["last_result"] = res
    out = np.zeros((B, SQ, D_MODEL), dtype=np.float32)
    for core in range(N_CORES):
        out[core // 2] += res.results[core]["out"].astype(np.float32)
    return out


if __name__ == "__main__":
    rng = np.random.default_rng(0)
    ins = {
        "x": rng.standard_normal((B, SQ, D_MODEL), dtype=np.float32),
        "cache_k": rng.standard_normal((B, N_HEADS, SKV, D_HEAD), dtype=np.float32),
        "cache_v": rng.standard_normal((B, N_HEADS, SKV, D_HEAD), dtype=np.float32),
        "w_qkv": rng.standard_normal((D_MODEL, 3 * D_MODEL), dtype=np.float32) * D_MODEL ** -0.5,
        "w_o": rng.standard_normal((D_MODEL, D_MODEL), dtype=np.float32) * D_MODEL ** -0.5,
    }
    out = kernel(**ins)
    print("out", out.shape, out.dtype, float(np.abs(out).max()))
